# revision 1
# baseline (speedup 1.0000x reference)
# Trainium2 Bass kernel for nn_Network_515396076038 (nms_detection / OICR-style loss).
#
# Strategy (8 NeuronCores, data-parallel over the N=4096 proposals):
#   - Each core owns NS = N/8 = 512 rois, streams its shard of fc7_roi /
#     fc7_frame / fc7_context from HBM (24 MB per core -> memory roofline),
#     transposes 128x128 chunks on the PE and computes the GEMM heads in
#     class-major layout [C, NS] (scores^T = W^T @ X^T accumulated over F).
#     The PE instruction stream is software-pipelined (transposes of chunk k
#     issue before the matmuls of chunk k-1) so the in-order PE never stalls
#     on the PSUM->SBUF copy of the transposed chunk.
#   - Per-class argmax over rois is computed locally (sel-mask trick: the
#     pseudo-GT box of each class is sel^T @ boxes on the PE - an exact
#     one-hot gather with no indexing); one AllGather ships each core's
#     per-class maxima + candidate boxes + softmax partial sums; every core
#     then reduces the 8 candidates itself. A final tiny AllReduce(add) sums
#     the refine-loss numerators/denominators. Work with no collective
#     dependency (refine-head log-softmax, transposes) is emitted between the
#     AllGather issue and its readback so it fills the collective latency.
#   - IoU, fg/bg assignment and the one-hot log-prob gather run in roi-major
#     layout [128, 2*C] with both supervision branches paired in one tile.
import sys

for _p in ("/opt/trn_rl_repo",):
    if _p not in sys.path:
        sys.path.append(_p)

import numpy as np

import concourse.bass as bass
import concourse.bass_isa as bass_isa
import concourse.mybir as mybir
import concourse.tile as tile
from concourse import bacc
from concourse.bass_utils import run_bass_kernel_spmd
from concourse.masks import make_identity

dt = mybir.dt
Alu = mybir.AluOpType
Act = mybir.ActivationFunctionType
AX = mybir.AxisListType

C = 20      # foreground classes
CR = C + 1  # refine head classes (background + C)
CA = C + 2 * CR  # stacked roi-head outputs: cls | r1 | r2 = 62


def _emit(nc, tc, aps, NS, F, n_cores):
    NB = NS // 128
    KT = F // 128
    group = [list(range(n_cores))]
    GW = 241  # AllGather row: vm1[20] vm2[21] boxes[160] z[20] s1[20]

    roi, frm, ctxm, w_all, w_det, b_all, boxes, isw, lab, loss = aps

    const = tc.alloc_tile_pool(name="const", bufs=1)
    st = tc.alloc_tile_pool(name="st", bufs=1)
    stp = tc.alloc_tile_pool(name="stp", bufs=2)
    natp = tc.alloc_tile_pool(name="natp", bufs=2)
    pst = tc.alloc_tile_pool(name="pst", bufs=3, space="PSUM")
    pss1 = tc.alloc_tile_pool(name="pss1", bufs=2, space="PSUM")
    pss2 = tc.alloc_tile_pool(name="pss2", bufs=1, space="PSUM")
    dp = tc.alloc_tile_pool(name="dp", bufs=1, space="DRAM")
    # psc is created last: it is the first pool released (LIFO pool stack)
    psc = tc.alloc_tile_pool(name="psc", bufs=1, space="PSUM")

    # ---------------- constants ----------------
    ident = const.tile([128, 128], dt.float32)
    make_identity(nc, ident)
    ones_col = const.tile([128, 1], dt.float32)
    nc.vector.memset(ones_col, 1.0)
    ones_row = const.tile([1, 128], dt.float32)
    nc.vector.memset(ones_row, 1.0)
    iota_i = const.tile([128, CR], dt.int32)
    nc.gpsimd.iota(iota_i, pattern=[[1, CR]], base=0, channel_multiplier=0)
    iota_f = const.tile([128, CR], dt.float32)
    nc.vector.tensor_copy(iota_f, iota_i)
    iota_m1k = const.tile([128, C], dt.float32)
    nc.vector.tensor_scalar_add(iota_m1k, iota_f[:, :C], -1000.0)

    w_all_sb = const.tile([128, KT, CA], dt.float32)
    nc.sync.dma_start(w_all_sb, w_all.rearrange("(ko p) c -> p ko c", p=128))
    w_det_sb = const.tile([128, KT, C], dt.float32)
    nc.sync.dma_start(w_det_sb, w_det.rearrange("(ko p) c -> p ko c", p=128))
    b_all_sb = const.tile([CA, 1], dt.float32)
    nc.sync.dma_start(b_all_sb, b_all[:, None])

    labrow_i = st.tile([1, C], dt.int32)
    nc.sync.dma_start(labrow_i, lab)
    labrow_f = st.tile([1, C], dt.float32)
    nc.vector.tensor_copy(labrow_f, labrow_i)
    mask_row = st.tile([1, 2 * C], dt.float32)
    nc.vector.tensor_scalar(mask_row[:, 0:C], labrow_f, 1.0, None, Alu.is_equal)
    nc.vector.tensor_copy(mask_row[:, C:2 * C], mask_row[:, 0:C])

    isw_row = st.tile([1, NS], dt.float32)
    nc.sync.dma_start(isw_row, isw[None, :])
    isw_col = st.tile([128, NB], dt.float32)
    nc.sync.dma_start(isw_col, isw.rearrange("(b p) -> p b", p=128))
    boxes_nat = st.tile([128, NB, 4], dt.float32)
    nc.sync.dma_start(boxes_nat, boxes.rearrange("(b p) x -> p b x", p=128))

    # ---------------- main GEMM phase (software-pipelined PE stream) --------
    # scoresA rows: [0:C] cls, [C:C+CR] r1, [C+CR:CA] r2 ; scoresB rows [0:C] det
    scoresA = psc.tile([128, NS], dt.float32)
    scoresB = psc.tile([128, NS], dt.float32)

    # inputs arrive host-transposed: roi/frm/ctx are [F, NS]; stream 2 MB
    # super-tiles of KS k-slices and run full-width (N=NS) matmuls.
    KS = min(8, KT)
    SK = KT // KS
    for sk in range(SK):
        ksl = bass.ts(sk, KS)
        t_roi = natp.tile([128, KS, NS], dt.float32, tag="roi")
        nc.sync.dma_start(t_roi, roi[:, ksl, :])
        t_frm = natp.tile([128, KS, NS], dt.float32, tag="frm")
        nc.sync.dma_start(t_frm, frm[:, ksl, :])
        t_ctx = natp.tile([128, KS, NS], dt.float32, tag="ctx")
        nc.sync.dma_start(t_ctx, ctxm[:, ksl, :])
        t_dif = natp.tile([128, KS, NS], dt.float32, tag="diff")
        for j in range(KS):
            nc.vector.tensor_sub(t_dif[:, j, :], t_frm[:, j, :], t_ctx[:, j, :])
        for j in range(KS):
            k = sk * KS + j
            nc.tensor.matmul(
                scoresA[0:CA, :], w_all_sb[:, k, :], t_roi[:, j, :],
                start=(k == 0), stop=(k == KT - 1),
            )
        for j in range(KS):
            k = sk * KS + j
            nc.tensor.matmul(
                scoresB[0:C, :], w_det_sb[:, k, :], t_dif[:, j, :],
                start=(k == 0), stop=(k == KT - 1),
            )

    # ---------------- class-major stats ----------------
    stk = st.tile([CA, NS], dt.float32)
    nc.scalar.activation(stk, scoresA[0:CA, :], Act.Identity, bias=b_all_sb)
    det_sb = st.tile([C, NS], dt.float32)
    nc.vector.tensor_copy(det_sb, scoresB[0:C, :])
    psc.release()
    # de-stack r1/r2 to partition base 0 (SBUF->SBUF DMA moves partitions)
    r1_sb = st.tile([CR, NS], dt.float32)
    nc.sync.dma_start(r1_sb, stk[C:C + CR, :])
    r2_sb = st.tile([CR, NS], dt.float32)
    nc.sync.dma_start(r2_sb, stk[C + CR:CA, :])

    exp_det = st.tile([C, NS], dt.float32)
    nc.scalar.activation(exp_det, det_sb, Act.Exp)
    zloc = st.tile([C, 1], dt.float32)
    nc.vector.reduce_sum(zloc, exp_det, axis=AX.X)
    prod_cd = st.tile([C, NS], dt.float32)
    nc.vector.tensor_mul(prod_cd, stk[0:C, :], exp_det)
    s1loc = st.tile([C, 1], dt.float32)
    nc.vector.reduce_sum(s1loc, prod_cd, axis=AX.X)

    exp_cls = st.tile([C, NS], dt.float32)
    nc.scalar.activation(exp_cls, stk[0:C, :], Act.Exp)
    exp_r1 = st.tile([CR, NS], dt.float32)
    nc.scalar.activation(exp_r1, r1_sb, Act.Exp)

    ps_s1 = pss1.tile([128, 512], dt.float32, tag="mm")
    nc.tensor.matmul(ps_s1[0:1, 0:NS], ones_col[0:C, :], exp_cls, start=True, stop=True)
    scls = st.tile([1, NS], dt.float32)
    nc.vector.tensor_copy(scls, ps_s1[0:1, 0:NS])
    ps_s2 = pss1.tile([128, 512], dt.float32, tag="mm")
    nc.tensor.matmul(ps_s2[0:1, 0:NS], ones_col[0:CR, :], exp_r1, start=True, stop=True)
    sr1 = st.tile([1, NS], dt.float32)
    nc.vector.tensor_copy(sr1, ps_s2[0:1, 0:NS])

    rb1 = st.tile([1, NS], dt.float32)
    nc.vector.reciprocal(rb1, scls)
    nc.vector.tensor_mul(rb1, rb1, isw_row)
    rb2 = st.tile([1, NS], dt.float32)
    nc.vector.reciprocal(rb2, sr1)
    nc.vector.tensor_mul(rb2, rb2, isw_row)

    ps_b1 = pss1.tile([128, 512], dt.float32, tag="mm")
    nc.tensor.matmul(ps_b1[0:C, 0:NS], ones_row[:, 0:C], rb1, start=True, stop=True)
    bc1 = st.tile([C, NS], dt.float32)
    nc.vector.tensor_copy(bc1, ps_b1[0:C, 0:NS])
    ps_b2 = pss1.tile([128, 512], dt.float32, tag="mm")
    nc.tensor.matmul(ps_b2[0:CR, 0:NS], ones_row[:, 0:CR], rb2, start=True, stop=True)
    bc2 = st.tile([CR, NS], dt.float32)
    nc.vector.tensor_copy(bc2, ps_b2[0:CR, 0:NS])

    p1 = st.tile([C, NS], dt.float32)
    nc.vector.tensor_mul(p1, exp_cls, exp_det)
    nc.vector.tensor_mul(p1, p1, bc1)
    q2 = st.tile([CR, NS], dt.float32)
    nc.vector.tensor_mul(q2, exp_r1, bc2)

    vm1 = st.tile([C, 1], dt.float32)
    nc.vector.reduce_max(vm1, p1, axis=AX.X)
    vm2 = st.tile([CR, 1], dt.float32)
    nc.vector.reduce_max(vm2, q2, axis=AX.X)

    # local per-class argmax boxes via sel-mask matmuls (no collective dep)
    sel1 = st.tile([C, NS], dt.float32)
    nc.vector.tensor_scalar(sel1, p1, vm1, None, Alu.is_equal)
    sel2 = st.tile([CR, NS], dt.float32)
    nc.vector.tensor_scalar(sel2, q2, vm2, None, Alu.is_equal)
    psq = pss2.tile([128, 64], dt.float32, tag="acc")
    for b in range(NB):
        bsl = bass.ts(b, 128)
        ptx = pst.tile([128, 256], dt.float32, tag="pt")
        nc.tensor.transpose(ptx[:, 0:C], sel1[:, bsl], ident[0:C, 0:C])
        nc.tensor.transpose(ptx[:, 32:32 + CR], sel2[:, bsl], ident[0:CR, 0:CR])
        sT = stp.tile([128, 64], dt.float32, tag="sT")
        nc.vector.tensor_copy(sT[:, 0:C], ptx[:, 0:C])
        nc.vector.tensor_copy(sT[:, 32:32 + CR], ptx[:, 32:32 + CR])
        nc.tensor.matmul(
            psq[0:4, 0:C], boxes_nat[:, b, :], sT[:, 0:C],
            start=(b == 0), stop=(b == NB - 1),
        )
        nc.tensor.matmul(
            psq[0:4, C:2 * C], boxes_nat[:, b, :], sT[:, 33:33 + C],
            start=(b == 0), stop=(b == NB - 1),
        )
    bc_sb = st.tile([4, 2 * C], dt.float32)
    nc.vector.tensor_copy(bc_sb, psq[0:4, 0:2 * C])

    # ---------------- G1: AllGather of all cross-core state ----------------
    g1_in = dp.tile([GW], dt.float32)
    g1_out = dp.tile([n_cores * GW], dt.float32)
    nc.sync.dma_start(g1_in[0:C], vm1[:, 0])
    nc.sync.dma_start(g1_in[C:C + CR], vm2[:, 0])
    nc.sync.dma_start(g1_in[41:201], bc_sb)
    nc.sync.dma_start(g1_in[201:221], zloc[:, 0])
    nc.sync.dma_start(g1_in[221:241], s1loc[:, 0])
    nc.gpsimd.collective_compute(
        "AllGather", Alu.bypass, replica_groups=group,
        ins=[g1_in.opt()], outs=[g1_out.opt()],
    )

    # ---- collective-independent prep, emitted here to fill G1 latency ----
    rts = st.tile([128, NB * 2 * CR], dt.float32)      # [.., b*42 + s*21 + c]
    for b in range(NB):
        bsl = bass.ts(b, 128)
        ptr = pst.tile([128, 256], dt.float32, tag="pt")
        nc.tensor.transpose(ptr[:, 0:CR], r1_sb[:, bsl], ident[0:CR, 0:CR])
        nc.tensor.transpose(ptr[:, CR:2 * CR], r2_sb[:, bsl], ident[0:CR, 0:CR])
        nc.vector.tensor_copy(rts[:, b * 2 * CR:(b + 1) * 2 * CR], ptr[:, 0:2 * CR])
    xs_all = st.tile([128, NB * 2 * CR], dt.float32)
    ssum_all = st.tile([128, 2 * NB], dt.float32)
    lse_all = st.tile([128, 2 * NB], dt.float32)
    for b in range(NB):
        for s in range(2):
            sl = slice((2 * b + s) * CR, (2 * b + s + 1) * CR)
            rmax = stp.tile([128, 1], dt.float32, tag="rmax")
            nc.vector.reduce_max(rmax, rts[:, sl], axis=AX.X)
            nc.vector.tensor_scalar(xs_all[:, sl], rts[:, sl], rmax, None, Alu.subtract)
    ex_all = st.tile([128, NB * 2 * CR], dt.float32)
    nc.scalar.activation(ex_all, xs_all, Act.Exp)
    for b in range(NB):
        for s in range(2):
            sl = slice((2 * b + s) * CR, (2 * b + s + 1) * CR)
            nc.vector.reduce_sum(ssum_all[:, 2 * b + s:2 * b + s + 1], ex_all[:, sl], axis=AX.X)
    nc.scalar.activation(lse_all, ssum_all, Act.Ln)
    for b in range(NB):
        for s in range(2):
            sl = slice((2 * b + s) * CR, (2 * b + s + 1) * CR)
            nc.vector.tensor_scalar(
                xs_all[:, sl], xs_all[:, sl], lse_all[:, 2 * b + s:2 * b + s + 1],
                None, Alu.subtract,
            )  # xs_all now holds log-probs
    ab_all = st.tile([128, NB], dt.float32)
    for b in range(NB):
        t1 = stp.tile([128, 1], dt.float32, tag="abt1")
        t2 = stp.tile([128, 1], dt.float32, tag="abt2")
        nc.vector.tensor_sub(t1, boxes_nat[:, b, 2:3], boxes_nat[:, b, 0:1])
        nc.vector.tensor_scalar_add(t1, t1, 1.0)
        nc.vector.tensor_sub(t2, boxes_nat[:, b, 3:4], boxes_nat[:, b, 1:2])
        nc.vector.tensor_scalar_add(t2, t2, 1.0)
        nc.vector.tensor_mul(ab_all[:, b:b + 1], t1, t2)

    # ---------------- G1 readback + cross-core combine ----------------
    g_sb = st.tile([n_cores, GW], dt.float32)
    nc.sync.dma_start(g_sb, g1_out.rearrange("(r w) -> r w", r=n_cores))
    vmx = st.tile([n_cores, 41], dt.float32)
    nc.gpsimd.partition_all_reduce(
        vmx, g_sb[:, 0:41], channels=n_cores, reduce_op=bass_isa.ReduceOp.max
    )
    selc = st.tile([n_cores, 41], dt.float32)
    nc.vector.tensor_tensor(selc, g_sb[:, 0:41], vmx, Alu.is_equal)
    masked = st.tile([n_cores, 160], dt.float32)
    mview = masked.rearrange("p (co s c) -> p co s c", co=4, s=2)
    gview = g_sb[:, 41:201].rearrange("p (co s c) -> p co s c", co=4, s=2)
    nc.vector.tensor_tensor(
        mview[:, :, 0, :], gview[:, :, 0, :],
        selc[:, None, 0:C].to_broadcast([n_cores, 4, C]), Alu.mult,
    )
    nc.vector.tensor_tensor(
        mview[:, :, 1, :], gview[:, :, 1, :],
        selc[:, None, CR:CR + C].to_broadcast([n_cores, 4, C]), Alu.mult,
    )
    ps_qr = pss1.tile([128, 512], dt.float32, tag="mm")
    nc.tensor.matmul(ps_qr[0:1, 0:160], ones_col[0:n_cores, :], masked,
                     start=True, stop=True)
    nc.tensor.matmul(ps_qr[0:1, 160:200], ones_col[0:n_cores, :], g_sb[:, 201:241],
                     start=True, stop=True)
    qzs = st.tile([1, 200], dt.float32)
    nc.vector.tensor_copy(qzs, ps_qr[0:1, 0:200])

    ps_q = pss1.tile([128, 512], dt.float32, tag="mm")
    nc.tensor.matmul(ps_q[:, 0:160], ones_row[0:1, :], qzs[:, 0:160],
                     start=True, stop=True)
    nc.tensor.matmul(ps_q[:, 160:200], ones_row[0:1, :], mask_row,
                     start=True, stop=True)
    Qall = st.tile([128, 200], dt.float32)
    nc.vector.tensor_copy(Qall, ps_q[:, 0:200])
    maskP = Qall[:, 160:200]
    maskP_m1 = st.tile([128, 2 * C], dt.float32)
    nc.vector.tensor_scalar_add(maskP_m1, maskP, -1.0)
    # paired (both supervisions) query areas [128, 40]
    aqp = st.tile([128, 2 * C], dt.float32)
    thp = st.tile([128, 2 * C], dt.float32)
    nc.vector.tensor_sub(aqp, Qall[:, 80:120], Qall[:, 0:40])
    nc.vector.tensor_scalar_add(aqp, aqp, 1.0)
    nc.vector.tensor_sub(thp, Qall[:, 120:160], Qall[:, 40:80])
    nc.vector.tensor_scalar_add(thp, thp, 1.0)
    nc.vector.tensor_mul(aqp, aqp, thp)

    # ---------------- per-block paired IoU / assignment / loss ----------------
    ps_l = pss2.tile([128, 64], dt.float32, tag="acc")
    for b in range(NB):
        bx1 = boxes_nat[:, b, 0:1]
        by1 = boxes_nat[:, b, 1:2]
        bx2 = boxes_nat[:, b, 2:3]
        by2 = boxes_nat[:, b, 3:4]
        xi1 = stp.tile([128, 2 * C], dt.float32, tag="xi1")
        nc.vector.tensor_scalar_max(xi1, Qall[:, 0:40], bx1)
        yi1 = stp.tile([128, 2 * C], dt.float32, tag="yi1")
        nc.vector.tensor_scalar_max(yi1, Qall[:, 40:80], by1)
        xi2 = stp.tile([128, 2 * C], dt.float32, tag="xi2")
        nc.vector.tensor_scalar_min(xi2, Qall[:, 80:120], bx2)
        yi2 = stp.tile([128, 2 * C], dt.float32, tag="yi2")
        nc.vector.tensor_scalar_min(yi2, Qall[:, 120:160], by2)
        nc.vector.tensor_sub(xi2, xi2, xi1)
        nc.vector.tensor_scalar(xi2, xi2, 1.0, 0.0, Alu.add, Alu.max)   # iw
        nc.vector.tensor_sub(yi2, yi2, yi1)
        nc.vector.tensor_scalar(yi2, yi2, 1.0, 0.0, Alu.add, Alu.max)   # ih
        inter = stp.tile([128, 2 * C], dt.float32, tag="inter")
        nc.vector.tensor_mul(inter, xi2, yi2)
        un = stp.tile([128, 2 * C], dt.float32, tag="un")
        nc.vector.tensor_scalar(un, aqp, ab_all[:, b:b + 1], None, Alu.add)
        nc.vector.tensor_sub(un, un, inter)
        nc.vector.reciprocal(un, un)
        ov = stp.tile([128, 2 * C], dt.float32, tag="ov")
        nc.vector.tensor_mul(ov, inter, un)
        nc.vector.tensor_mul(ov, ov, maskP)
        nc.vector.tensor_add(ov, ov, maskP_m1)

        stats_b = stp.tile([128, 4], dt.float32, tag="stats")
        for s in range(2):
            ovs = ov[:, s * C:(s + 1) * C]
            mo = stp.tile([128, 1], dt.float32, tag="mo")
            nc.vector.reduce_max(mo, ovs, axis=AX.X)
            meq = stp.tile([128, C], dt.float32, tag="meq")
            nc.vector.tensor_scalar(meq, ovs, mo, None, Alu.is_equal)
            nc.vector.tensor_mul(meq, meq, iota_m1k)
            gt = stp.tile([128, 1], dt.float32, tag="gt")
            nc.vector.tensor_reduce(gt, meq, axis=AX.X, op=Alu.min)
            nc.vector.tensor_scalar_add(gt, gt, 1001.0)   # argmax + 1

            fg = stp.tile([128, 1], dt.float32, tag="fg")
            nc.vector.tensor_scalar(fg, mo, 0.5, None, Alu.is_gt)
            bg = stp.tile([128, 1], dt.float32, tag="bg")
            nc.vector.tensor_scalar(bg, mo, 0.1, None, Alu.is_ge)
            bgt = stp.tile([128, 1], dt.float32, tag="bgt")
            nc.vector.tensor_scalar(bgt, mo, 0.5, None, Alu.is_lt)
            nc.vector.tensor_mul(bg, bg, bgt)
            keep = stp.tile([128, 1], dt.float32, tag="keep")
            nc.vector.tensor_add(keep, fg, bg)
            col = stp.tile([128, 1], dt.float32, tag="col")
            nc.vector.tensor_mul(col, gt, fg)   # fg ? gt+1 : 0
            oh = stp.tile([128, CR], dt.float32, tag="oh")
            nc.vector.tensor_scalar(oh, iota_f, col, None, Alu.is_equal)

            lp = xs_all[:, (2 * b + s) * CR:(2 * b + s + 1) * CR]
            nc.vector.tensor_mul(oh, oh, lp)
            lpsel = stp.tile([128, 1], dt.float32, tag="lpsel")
            nc.vector.reduce_sum(lpsel, oh, axis=AX.X)

            w = stp.tile([128, 1], dt.float32, tag="w")
            nc.vector.tensor_mul(w, keep, isw_col[:, b:b + 1])
            nc.vector.tensor_mul(stats_b[:, 2 * s:2 * s + 1], w, lpsel)
            nc.vector.tensor_copy(stats_b[:, 2 * s + 1:2 * s + 2], keep)
        nc.tensor.matmul(
            ps_l[0:4, 0:1], stats_b, ones_col,
            start=(b == 0), stop=(b == NB - 1),
        )
    lsum = st.tile([4, 1], dt.float32)
    nc.vector.tensor_copy(lsum, ps_l[0:4, 0:1])

    # ---------------- R3: AllReduce(add) of loss partials ----------------
    cc3_in = dp.tile([4], dt.float32)
    cc3_out = dp.tile([4], dt.float32)
    nc.sync.dma_start(cc3_in, lsum[:, 0])
    nc.gpsimd.collective_compute(
        "AllReduce", Alu.add, replica_groups=group,
        ins=[cc3_in.opt()], outs=[cc3_out.opt()],
    )
    l4 = st.tile([1, 4], dt.float32)
    nc.sync.dma_start(l4, cc3_out[None, :])

    # ---------------- final scalar (row layout, partition 0) ----------------
    zrow = qzs[:, 160:180]
    s1row = qzs[:, 180:200]
    zinv = st.tile([1, C], dt.float32)
    nc.vector.reciprocal(zinv, zrow)
    dcs = st.tile([1, C], dt.float32)
    nc.vector.tensor_mul(dcs, s1row, zinv)
    hv = st.tile([1, C], dt.float32)
    nc.vector.tensor_mul(hv, labrow_f, dcs)
    nc.scalar.activation(hv, hv, Act.Relu, bias=1.0, scale=-1.0)  # relu(1-lab*dcs)
    h = st.tile([1, 1], dt.float32)
    nc.vector.reduce_sum(h, hv, axis=AX.X)

    inv1 = st.tile([1, 1], dt.float32)
    nc.vector.reciprocal(inv1, l4[:, 1:2])
    nc.vector.tensor_mul(inv1, inv1, l4[:, 0:1])
    inv2 = st.tile([1, 1], dt.float32)
    nc.vector.reciprocal(inv2, l4[:, 3:4])
    nc.vector.tensor_mul(inv2, inv2, l4[:, 2:3])
    tot = st.tile([1, 1], dt.float32)
    nc.vector.tensor_add(tot, inv1, inv2)
    nc.scalar.mul(tot, tot, -0.1)
    nc.scalar.mul(h, h, 1.0 / C)
    nc.vector.tensor_add(tot, tot, h)
    nc.sync.dma_start(loss, tot)

    for pool in (dp, pss2, pss1, pst, natp, stp, st, const):
        pool.release()


def build_program(NS=512, F=4096, n_cores=8):
    nc = bacc.Bacc(
        "TRN2", target_bir_lowering=False, debug=False, num_devices=n_cores
    )
    roi = nc.dram_tensor("roi", [128, F // 128, NS], dt.float32, kind="ExternalInput").ap()
    frm = nc.dram_tensor("frm", [128, F // 128, NS], dt.float32, kind="ExternalInput").ap()
    ctxm = nc.dram_tensor("ctxm", [128, F // 128, NS], dt.float32, kind="ExternalInput").ap()
    w_all = nc.dram_tensor("w_all", [F, CA], dt.float32, kind="ExternalInput").ap()
    w_det = nc.dram_tensor("w_det", [F, C], dt.float32, kind="ExternalInput").ap()
    b_all = nc.dram_tensor("b_all", [CA], dt.float32, kind="ExternalInput").ap()
    boxes = nc.dram_tensor("boxes", [NS, 4], dt.float32, kind="ExternalInput").ap()
    isw = nc.dram_tensor("isw", [NS], dt.float32, kind="ExternalInput").ap()
    lab = nc.dram_tensor("lab", [1, C], dt.int32, kind="ExternalInput").ap()
    loss = nc.dram_tensor("loss", [1, 1], dt.float32, kind="ExternalOutput").ap()
    aps = (roi, frm, ctxm, w_all, w_det, b_all, boxes, isw, lab, loss)
    with tile.TileContext(nc) as tc:
        _emit(nc, tc, aps, NS, F, n_cores)
    nc.compile()
    return nc


def make_in_maps(inputs, NS, n_cores):
    f32 = np.float32
    w_all = np.ascontiguousarray(
        np.concatenate(
            [np.asarray(inputs["W_cls"]), np.asarray(inputs["W_r1"]),
             np.asarray(inputs["W_r2"])], axis=1
        ), f32
    )
    b_all = np.ascontiguousarray(
        np.concatenate(
            [np.asarray(inputs["b_cls"]), np.asarray(inputs["b_r1"]),
             np.asarray(inputs["b_r2"])]
        ), f32
    )
    w_det = np.ascontiguousarray(np.asarray(inputs["W_det"]), f32)
    boxes = np.ascontiguousarray(np.asarray(inputs["ss_boxes"])[:, 1:5], f32)
    isw = np.ascontiguousarray(np.asarray(inputs["IS_weight"])[:, 0], f32)
    lab = np.ascontiguousarray(np.asarray(inputs["image_level_label"]), np.int32)
    roi = np.asarray(inputs["fc7_roi"], f32).T
    frm = np.asarray(inputs["fc7_frame"], f32).T
    ctxm = np.asarray(inputs["fc7_context"], f32).T
    F = roi.shape[0]

    def _pack(a, sl):
        # [F, NS] slice -> [128, KT, NS]: 16KB-contiguous per-partition runs
        return np.ascontiguousarray(
            a[:, sl].reshape(F // 128, 128, -1).transpose(1, 0, 2))

    in_maps = []
    for c in range(n_cores):
        sl = slice(c * NS, (c + 1) * NS)
        in_maps.append({
            "roi": _pack(roi, sl),
            "frm": _pack(frm, sl),
            "ctxm": _pack(ctxm, sl),
            "w_all": w_all, "w_det": w_det, "b_all": b_all,
            "boxes": boxes[sl], "isw": isw[sl], "lab": lab,
        })
    return in_maps


_PROG_CACHE = {}


def _get_prog(NS, F, n_cores):
    key = (NS, F, n_cores)
    if key not in _PROG_CACHE:
        _PROG_CACHE[key] = build_program(NS, F, n_cores)
    return _PROG_CACHE[key]


def kernel(**inputs):
    n_cores = 8
    N, F = inputs["fc7_roi"].shape
    NS = N // n_cores
    prog = _get_prog(NS, F, n_cores)
    in_maps = make_in_maps(inputs, NS, n_cores)
    res = run_bass_kernel_spmd(prog, in_maps, list(range(n_cores))).results
    return np.float32(res[0]["loss"].reshape(()))



# revision 12
# speedup vs baseline: 1.5291x; 1.5291x over previous
# Trainium2 Bass kernel for nn_Network_515396076038 (nms_detection / OICR-style loss).
#
# v2 strategy (8 NeuronCores, data-parallel over the N=4096 proposals):
#   - Inputs stream in bf16 (host-cast): 13 MB/core instead of 24 MB, and the
#     PE runs bf16 matmuls at 1 cycle/row instead of fp32's 4. fc7 shards are
#     host-packed to [128, KT, NS] so every big DMA is 128 contiguous
#     per-partition runs (no descriptor blowup). det head uses [W_det | -W_det]
#     so frame/context accumulate into one PSUM bank with no vector subtract.
#   - The r2 refine head GEMM + both heads' roi-major log-softmax run AFTER
#     the AllGather trigger (t_roi stays resident in SBUF), filling the
#     collective's latency window; only cls/det/r1 are on the critical path
#     to the collective.
#   - One AllGather ships per-class argmax candidates + boxes + softmax
#     partial sums. The IoU / fg-bg assignment / loss phase is batched over
#     all NB roi-blocks and both supervisions in single wide vector ops.
#   - No second collective: each core outputs its hinge term plus per-shard
#     loss partial sums; kernel() adds the 8×16 partials on host (the
#     gather/unshard step).
import sys

for _p in ("/opt/trn_rl_repo",):
    if _p not in sys.path:
        sys.path.append(_p)

import ml_dtypes
import numpy as np

import concourse.bass as bass
import concourse.bass_isa as bass_isa
import concourse.mybir as mybir
import concourse.tile as tile
from concourse import bacc
from concourse.bass_utils import run_bass_kernel_spmd
from concourse.masks import make_identity

dt = mybir.dt
Alu = mybir.AluOpType
Act = mybir.ActivationFunctionType
AX = mybir.AxisListType

C = 20       # foreground classes
CR = C + 1   # refine head classes (background + C)
CW = C + CR  # stacked critical-path roi heads: cls | r1 = 41


def _emit(nc, tc, aps, NS, F, n_cores):
    NB = NS // 128   # 4 roi blocks
    KT = F // 128    # 32 contraction slices
    KH = KT // 2     # half-chunk for DMA/compute interleave
    NP = NB * 2      # (block, supervision) pairs, index b*2+s
    group = [list(range(n_cores))]
    GW = 241  # AllGather row: vm1[20] vm2[21] boxes[160] z[20] s1[20]

    (roi, frm, ctxm, w_a, w_d2, w_r2, b_a, b_r2, bxw, isw, lab, out) = aps

    const = tc.alloc_tile_pool(name="const", bufs=1)
    st = tc.alloc_tile_pool(name="st", bufs=1)
    stp = tc.alloc_tile_pool(name="stp", bufs=2)
    pst = tc.alloc_tile_pool(name="pst", bufs=2, space="PSUM")
    pss = tc.alloc_tile_pool(name="pss", bufs=2, space="PSUM")
    psa = tc.alloc_tile_pool(name="psa", bufs=1, space="PSUM")
    dp = tc.alloc_tile_pool(name="dp", bufs=1, space="DRAM")
    psc = tc.alloc_tile_pool(name="psc", bufs=1, space="PSUM")

    # ---------------- weights + fc7 chunk DMAs (issue order = arrival order)
    w_a_sb = const.tile([128, KT, CW], dt.bfloat16)
    nc.sync.dma_start(w_a_sb, w_a)
    w_d2_sb = const.tile([128, KT, 2 * C], dt.bfloat16)
    nc.sync.dma_start(w_d2_sb, w_d2)
    b_a_sb = const.tile([CW, 1], dt.float32)
    nc.sync.dma_start(b_a_sb, b_a[:, None])
    b_r2_sb = const.tile([CR, 1], dt.float32)
    nc.sync.dma_start(b_r2_sb, b_r2[:, None])
    bxw_sb = st.tile([128, NB, 5], dt.float32)
    nc.sync.dma_start(bxw_sb, bxw)
    isw_row = st.tile([1, NS], dt.float32)
    nc.sync.dma_start(isw_row, isw[None, :])
    labrow_i = st.tile([1, C], dt.int32)
    nc.sync.dma_start(labrow_i, lab)
    t_roi1 = st.tile([128, KH, NS], dt.bfloat16)
    nc.sync.dma_start(t_roi1, roi[:, 0:KH, :])
    t_frm1 = st.tile([128, KH, NS], dt.bfloat16)
    nc.sync.dma_start(t_frm1, frm[:, 0:KH, :])
    t_ctx1 = st.tile([128, KH, NS], dt.bfloat16)
    nc.sync.dma_start(t_ctx1, ctxm[:, 0:KH, :])
    t_roi2 = st.tile([128, KH, NS], dt.bfloat16)
    nc.sync.dma_start(t_roi2, roi[:, KH:KT, :])
    t_frm2 = st.tile([128, KH, NS], dt.bfloat16)
    nc.sync.dma_start(t_frm2, frm[:, KH:KT, :])
    t_ctx2 = st.tile([128, KH, NS], dt.bfloat16)
    nc.sync.dma_start(t_ctx2, ctxm[:, KH:KT, :])
    w_r2_sb = const.tile([128, KT, CR], dt.bfloat16)
    nc.sync.dma_start(w_r2_sb, w_r2)
    boxes_nat = bxw_sb[:, :, 0:4]
    isw_col = bxw_sb[:, :, 4:5]     # [128, NB, 1]

    # ---------------- constants ----------------
    ident = const.tile([128, 128], dt.float32)
    make_identity(nc, ident)
    ones_col = const.tile([128, 1], dt.float32)
    nc.vector.memset(ones_col, 1.0)
    ones_row = const.tile([1, 128], dt.float32)
    nc.vector.memset(ones_row, 1.0)
    iota_i = const.tile([128, CR], dt.int32)
    nc.gpsimd.iota(iota_i, pattern=[[1, CR]], base=0, channel_multiplier=0)
    iota_f = const.tile([128, CR], dt.float32)
    nc.vector.tensor_copy(iota_f, iota_i)
    iota_m1k = const.tile([128, C], dt.float32)
    nc.vector.tensor_scalar_add(iota_m1k, iota_f[:, :C], -1000.0)

    labrow_f = st.tile([1, C], dt.float32)
    nc.vector.tensor_copy(labrow_f, labrow_i)
    mask_row = st.tile([1, 2 * C], dt.float32)
    nc.vector.tensor_scalar(mask_row[:, 0:C], labrow_f, 1.0, None, Alu.is_equal)
    nc.vector.tensor_copy(mask_row[:, C:2 * C], mask_row[:, 0:C])

    # ---------------- main GEMM (bf16, PE trails the DMA stream) -----------
    scoresA = psc.tile([128, NS], dt.float32)   # rows 0:CW = cls | r1
    scoresB = psc.tile([128, NS], dt.float32)   # rows 0:C  = det (frm - ctx)
    scoresR = psc.tile([128, NS], dt.float32)   # rows 0:CR = r2 (deferred)
    for k in range(KH):
        nc.tensor.matmul(scoresA[0:CW, :], w_a_sb[:, k, :], t_roi1[:, k, :],
                         start=(k == 0), stop=False)
    for k in range(KH):
        nc.tensor.matmul(scoresB[0:C, :], w_d2_sb[:, k, 0:C], t_frm1[:, k, :],
                         start=(k == 0), stop=False)
    for k in range(KH):
        nc.tensor.matmul(scoresB[0:C, :], w_d2_sb[:, k, C:2 * C], t_ctx1[:, k, :],
                         start=False, stop=False)
    for k in range(KH):
        nc.tensor.matmul(scoresA[0:CW, :], w_a_sb[:, KH + k, :], t_roi2[:, k, :],
                         start=False, stop=(k == KH - 1))
    for k in range(KH):
        nc.tensor.matmul(scoresB[0:C, :], w_d2_sb[:, KH + k, 0:C], t_frm2[:, k, :],
                         start=False, stop=False)
    for k in range(KH):
        nc.tensor.matmul(scoresB[0:C, :], w_d2_sb[:, KH + k, C:2 * C], t_ctx2[:, k, :],
                         start=False, stop=(k == KH - 1))

    # ---------------- class-major stats (critical path to the AllGather) ---
    stk = st.tile([CW, NS], dt.float32)
    nc.scalar.activation(stk, scoresA[0:CW, :], Act.Identity, bias=b_a_sb)
    det_sb = st.tile([C, NS], dt.float32)
    nc.vector.tensor_copy(det_sb, scoresB[0:C, :])
    # de-stack r1 to partition base 0 (SBUF->SBUF DMA moves partitions)
    r1_sb = st.tile([CR, NS], dt.float32)
    nc.sync.dma_start(r1_sb, stk[C:CW, :])

    exp_det = st.tile([C, NS], dt.float32)
    nc.scalar.activation(exp_det, det_sb, Act.Exp)
    zloc = st.tile([C, 1], dt.float32)
    nc.vector.reduce_sum(zloc, exp_det, axis=AX.X)
    prod_cd = st.tile([C, NS], dt.float32)
    nc.vector.tensor_mul(prod_cd, stk[0:C, :], exp_det)
    s1loc = st.tile([C, 1], dt.float32)
    nc.vector.reduce_sum(s1loc, prod_cd, axis=AX.X)

    exp_cls = st.tile([C, NS], dt.float32)
    nc.scalar.activation(exp_cls, stk[0:C, :], Act.Exp)
    exp_r1 = st.tile([CR, NS], dt.float32)
    nc.scalar.activation(exp_r1, r1_sb, Act.Exp)

    ps_s1 = pss.tile([128, 512], dt.float32, tag="mm")
    nc.tensor.matmul(ps_s1[0:1, 0:NS], ones_col[0:C, :], exp_cls, start=True, stop=True)
    scls = st.tile([1, NS], dt.float32)
    nc.vector.tensor_copy(scls, ps_s1[0:1, 0:NS])
    ps_s2 = pss.tile([128, 512], dt.float32, tag="mm")
    nc.tensor.matmul(ps_s2[0:1, 0:NS], ones_col[0:CR, :], exp_r1, start=True, stop=True)
    sr1 = st.tile([1, NS], dt.float32)
    nc.vector.tensor_copy(sr1, ps_s2[0:1, 0:NS])

    rb1 = st.tile([1, NS], dt.float32)
    nc.vector.reciprocal(rb1, scls)
    nc.vector.tensor_mul(rb1, rb1, isw_row)
    rb2 = st.tile([1, NS], dt.float32)
    nc.vector.reciprocal(rb2, sr1)
    nc.vector.tensor_mul(rb2, rb2, isw_row)

    ps_b1 = pss.tile([128, 512], dt.float32, tag="mm")
    nc.tensor.matmul(ps_b1[0:C, 0:NS], ones_row[:, 0:C], rb1, start=True, stop=True)
    bc1 = st.tile([C, NS], dt.float32)
    nc.vector.tensor_copy(bc1, ps_b1[0:C, 0:NS])
    ps_b2 = pss.tile([128, 512], dt.float32, tag="mm")
    nc.tensor.matmul(ps_b2[0:CR, 0:NS], ones_row[:, 0:CR], rb2, start=True, stop=True)
    bc2 = st.tile([CR, NS], dt.float32)
    nc.vector.tensor_copy(bc2, ps_b2[0:CR, 0:NS])

    p1 = st.tile([C, NS], dt.float32)
    nc.vector.tensor_mul(p1, exp_cls, exp_det)
    nc.vector.tensor_mul(p1, p1, bc1)
    q2 = st.tile([CR, NS], dt.float32)
    nc.vector.tensor_mul(q2, exp_r1, bc2)

    vm1 = st.tile([C, 1], dt.float32)
    nc.vector.reduce_max(vm1, p1, axis=AX.X)
    vm2 = st.tile([CR, 1], dt.float32)
    nc.vector.reduce_max(vm2, q2, axis=AX.X)

    # local per-class argmax boxes via sel-mask matmuls (exact one-hot gather)
    sel1 = st.tile([C, NS], dt.float32)
    nc.vector.tensor_scalar(sel1, p1, vm1, None, Alu.is_equal)
    sel2 = st.tile([CR, NS], dt.float32)
    nc.vector.tensor_scalar(sel2, q2, vm2, None, Alu.is_equal)
    psq = psa.tile([128, 512], dt.float32, tag="acc")
    for b in range(NB):
        bsl = bass.ts(b, 128)
        ptx = pst.tile([128, 64], dt.float32, tag="pt")
        nc.tensor.transpose(ptx[:, 0:C], sel1[:, bsl], ident[0:C, 0:C])
        nc.tensor.transpose(ptx[:, 32:32 + CR], sel2[:, bsl], ident[0:CR, 0:CR])
        sT = stp.tile([128, 64], dt.float32, tag="sT")
        nc.vector.tensor_copy(sT[:, 0:C], ptx[:, 0:C])
        nc.vector.memset(sT[:, C:32], 0.0)
        nc.vector.tensor_copy(sT[:, 32:32 + CR], ptx[:, 32:32 + CR])
        nc.tensor.matmul(
            psq[0:4, 0:53], boxes_nat[:, b, :], sT[:, 0:53],
            start=(b == 0), stop=(b == NB - 1),
        )  # cols 0:20 = sup1 boxes, 33:53 = sup2 boxes, rest junk
    bc_sb = st.tile([4, 2 * C], dt.float32)
    nc.vector.tensor_copy(bc_sb[:, 0:C], psq[0:4, 0:C])
    nc.vector.tensor_copy(bc_sb[:, C:2 * C], psq[0:4, 33:33 + C])

    # ---------------- G1: AllGather of all cross-core state ----------------
    g1_in = dp.tile([GW], dt.float32)
    g1_out = dp.tile([n_cores * GW], dt.float32)
    nc.sync.dma_start(g1_in[0:C], vm1[:, 0])
    nc.sync.dma_start(g1_in[C:C + CR], vm2[:, 0])
    nc.sync.dma_start(g1_in[41:201], bc_sb)
    nc.sync.dma_start(g1_in[201:221], zloc[:, 0])
    nc.sync.dma_start(g1_in[221:241], s1loc[:, 0])
    nc.gpsimd.collective_compute(
        "AllGather", Alu.bypass, replica_groups=group,
        ins=[g1_in.opt()], outs=[g1_out.opt()],
    )

    # ---- collective-latency filler: r2 GEMM + roi-major log-softmax -------
    for k in range(KH):
        nc.tensor.matmul(scoresR[0:CR, :], w_r2_sb[:, k, :], t_roi1[:, k, :],
                         start=(k == 0), stop=False)
    for k in range(KH):
        nc.tensor.matmul(scoresR[0:CR, :], w_r2_sb[:, KH + k, :], t_roi2[:, k, :],
                         start=False, stop=(k == KH - 1))
    r2_sb = st.tile([CR, NS], dt.float32)
    nc.scalar.activation(r2_sb, scoresR[0:CR, :], Act.Identity, bias=b_r2_sb)

    rts = st.tile([128, NP, CR], dt.float32)     # [.., b*2+s, c]
    for b in range(NB):
        bsl = bass.ts(b, 128)
        ptr = pst.tile([128, 64], dt.float32, tag="pt")
        nc.tensor.transpose(ptr[:, 0:CR], r1_sb[:, bsl], ident[0:CR, 0:CR])
        nc.tensor.transpose(ptr[:, CR:2 * CR], r2_sb[:, bsl], ident[0:CR, 0:CR])
        nc.vector.tensor_copy(rts[:, 2 * b, :], ptr[:, 0:CR])
        nc.vector.tensor_copy(rts[:, 2 * b + 1, :], ptr[:, CR:2 * CR])
    rmax = st.tile([128, NP, 1], dt.float32)
    nc.vector.reduce_max(rmax, rts, axis=AX.X)
    xs = st.tile([128, NP, CR], dt.float32)      # becomes log-probs
    nc.vector.tensor_tensor(
        xs, rts, rmax.to_broadcast([128, NP, CR]), Alu.subtract)
    ex = st.tile([128, NP, CR], dt.float32)
    nc.scalar.activation(ex, xs, Act.Exp)
    ssum = st.tile([128, NP, 1], dt.float32)
    nc.vector.reduce_sum(ssum, ex, axis=AX.X)
    lse = st.tile([128, NP, 1], dt.float32)
    nc.scalar.activation(lse, ssum, Act.Ln)
    nc.vector.tensor_tensor(
        xs, xs, lse.to_broadcast([128, NP, CR]), Alu.subtract)

    # roi box areas [128, NB, 1]
    ab_all = st.tile([128, NB, 1], dt.float32)
    tw = st.tile([128, NB, 1], dt.float32)
    nc.vector.tensor_tensor(ab_all, bxw_sb[:, :, 2:3], bxw_sb[:, :, 0:1], Alu.subtract)
    nc.vector.tensor_scalar_add(ab_all, ab_all, 1.0)
    nc.vector.tensor_tensor(tw, bxw_sb[:, :, 3:4], bxw_sb[:, :, 1:2], Alu.subtract)
    nc.vector.tensor_scalar_add(tw, tw, 1.0)
    nc.vector.tensor_mul(ab_all, ab_all, tw)

    # class mask broadcast down the partitions (local; overlaps collective)
    ps_m = pss.tile([128, 512], dt.float32, tag="mm")
    nc.tensor.matmul(ps_m[:, 0:2 * C], ones_row[0:1, :], mask_row, start=True, stop=True)
    maskP = st.tile([128, 2 * C], dt.float32)
    nc.vector.tensor_copy(maskP, ps_m[:, 0:2 * C])
    maskP_m1 = st.tile([128, 2 * C], dt.float32)
    nc.vector.tensor_scalar_add(maskP_m1, maskP, -1.0)

    # ---------------- G1 readback + cross-core combine ----------------
    g_sb = st.tile([n_cores, GW], dt.float32)
    nc.sync.dma_start(g_sb, g1_out.rearrange("(r w) -> r w", r=n_cores))
    vmx = st.tile([n_cores, 41], dt.float32)
    nc.gpsimd.partition_all_reduce(
        vmx, g_sb[:, 0:41], channels=n_cores, reduce_op=bass_isa.ReduceOp.max
    )
    selc = st.tile([n_cores, 41], dt.float32)
    nc.vector.tensor_tensor(selc, g_sb[:, 0:41], vmx, Alu.is_equal)
    masked = st.tile([n_cores, 160], dt.float32)
    mview = masked.rearrange("p (co s c) -> p co s c", co=4, s=2)
    gview = g_sb[:, 41:201].rearrange("p (co s c) -> p co s c", co=4, s=2)
    nc.vector.tensor_tensor(
        mview[:, :, 0, :], gview[:, :, 0, :],
        selc[:, None, 0:C].to_broadcast([n_cores, 4, C]), Alu.mult,
    )
    nc.vector.tensor_tensor(
        mview[:, :, 1, :], gview[:, :, 1, :],
        selc[:, None, CR:CR + C].to_broadcast([n_cores, 4, C]), Alu.mult,
    )
    ps_qr = pss.tile([128, 512], dt.float32, tag="mm")
    nc.tensor.matmul(ps_qr[0:1, 0:160], ones_col[0:n_cores, :], masked,
                     start=True, stop=True)
    nc.tensor.matmul(ps_qr[0:1, 160:200], ones_col[0:n_cores, :], g_sb[:, 201:241],
                     start=True, stop=True)
    qzs = st.tile([1, 200], dt.float32)
    nc.vector.tensor_copy(qzs, ps_qr[0:1, 0:200])

    ps_q = pss.tile([128, 512], dt.float32, tag="mm")
    nc.tensor.matmul(ps_q[:, 0:160], ones_row[0:1, :], qzs[:, 0:160],
                     start=True, stop=True)
    Qall = st.tile([128, 160], dt.float32)
    nc.vector.tensor_copy(Qall, ps_q[:, 0:160])
    # paired (both supervisions) query areas [128, 2C]
    aqp = st.tile([128, 2 * C], dt.float32)
    thp = st.tile([128, 2 * C], dt.float32)
    nc.vector.tensor_sub(aqp, Qall[:, 80:120], Qall[:, 0:40])
    nc.vector.tensor_scalar_add(aqp, aqp, 1.0)
    nc.vector.tensor_sub(thp, Qall[:, 120:160], Qall[:, 40:80])
    nc.vector.tensor_scalar_add(thp, thp, 1.0)
    nc.vector.tensor_mul(aqp, aqp, thp)

    # ---------------- batched IoU / assignment / loss over all blocks ------
    W2 = 2 * C
    def qb(lo):   # Qall coord block [128, 1, 2C] -> [128, NB, 2C]
        return Qall[:, None, lo:lo + W2].to_broadcast([128, NB, W2])
    def bb(i):    # per-block box coord [128, NB, 1] -> [128, NB, 2C]
        return boxes_nat[:, :, i:i + 1].to_broadcast([128, NB, W2])

    xi1 = st.tile([128, NB, W2], dt.float32)
    nc.vector.tensor_tensor(xi1, qb(0), bb(0), Alu.max)
    yi1 = st.tile([128, NB, W2], dt.float32)
    nc.vector.tensor_tensor(yi1, qb(40), bb(1), Alu.max)
    xi2 = st.tile([128, NB, W2], dt.float32)
    nc.vector.tensor_tensor(xi2, qb(80), bb(2), Alu.min)
    yi2 = st.tile([128, NB, W2], dt.float32)
    nc.vector.tensor_tensor(yi2, qb(120), bb(3), Alu.min)
    nc.vector.tensor_tensor(xi2, xi2, xi1, Alu.subtract)
    nc.vector.tensor_scalar(xi2, xi2, 1.0, 0.0, Alu.add, Alu.max)   # iw
    nc.vector.tensor_tensor(yi2, yi2, yi1, Alu.subtract)
    nc.vector.tensor_scalar(yi2, yi2, 1.0, 0.0, Alu.add, Alu.max)   # ih
    inter = st.tile([128, NB, W2], dt.float32)
    nc.vector.tensor_mul(inter, xi2, yi2)
    un = st.tile([128, NB, W2], dt.float32)
    nc.vector.tensor_tensor(
        un, aqp[:, None, :].to_broadcast([128, NB, W2]),
        ab_all.to_broadcast([128, NB, W2]), Alu.add)
    nc.vector.tensor_tensor(un, un, inter, Alu.subtract)
    nc.vector.reciprocal(un, un)
    ov = st.tile([128, NB, W2], dt.float32)
    nc.vector.tensor_mul(ov, inter, un)
    nc.vector.tensor_tensor(
        ov, ov, maskP[:, None, :].to_broadcast([128, NB, W2]), Alu.mult)
    nc.vector.tensor_tensor(
        ov, ov, maskP_m1[:, None, :].to_broadcast([128, NB, W2]), Alu.add)

    ovp = ov.rearrange("p b (s c) -> p (b s) c", s=2)   # [128, NP, C]
    mo = st.tile([128, NP, 1], dt.float32)
    nc.vector.reduce_max(mo, ovp, axis=AX.X)
    meq = st.tile([128, NP, C], dt.float32)
    nc.vector.tensor_tensor(
        meq, ovp, mo.to_broadcast([128, NP, C]), Alu.is_equal)
    nc.vector.tensor_tensor(
        meq, meq, iota_m1k[:, None, :].to_broadcast([128, NP, C]), Alu.mult)
    gt = st.tile([128, NP, 1], dt.float32)
    nc.vector.tensor_reduce(gt, meq, axis=AX.X, op=Alu.min)
    nc.vector.tensor_scalar_add(gt, gt, 1001.0)          # argmax + 1

    fg = st.tile([128, NP, 1], dt.float32)
    nc.vector.tensor_scalar(fg, mo, 0.5, None, Alu.is_gt)
    bg = st.tile([128, NP, 1], dt.float32)
    nc.vector.tensor_scalar(bg, mo, 0.1, None, Alu.is_ge)
    bgt = st.tile([128, NP, 1], dt.float32)
    nc.vector.tensor_scalar(bgt, mo, 0.5, None, Alu.is_lt)
    nc.vector.tensor_mul(bg, bg, bgt)
    keep = st.tile([128, NP, 1], dt.float32)
    nc.vector.tensor_add(keep, fg, bg)
    col = st.tile([128, NP, 1], dt.float32)
    nc.vector.tensor_mul(col, gt, fg)                    # fg ? argmax+1 : 0
    oh = st.tile([128, NP, CR], dt.float32)
    nc.vector.tensor_tensor(
        oh, iota_f[:, None, :].to_broadcast([128, NP, CR]),
        col.to_broadcast([128, NP, CR]), Alu.is_equal)
    nc.vector.tensor_mul(oh, oh, xs)                     # one-hot · log-probs
    lpsel = st.tile([128, NP, 1], dt.float32)
    nc.vector.reduce_sum(lpsel, oh, axis=AX.X)

    stats = st.tile([128, 16], dt.float32)               # wl[8] | keep[8]
    wv = stats[:, 0:NP].rearrange("p (b s) -> p b s", s=2)
    kv = keep.rearrange("p (b s) o -> p b (s o)", s=2)   # [128, NB, 2]
    nc.vector.tensor_tensor(wv, kv, isw_col.to_broadcast([128, NB, 2]), Alu.mult)
    nc.vector.tensor_mul(
        stats[:, 0:NP], stats[:, 0:NP],
        lpsel.rearrange("p n o -> p (n o)"))
    nc.vector.tensor_copy(stats[:, NP:2 * NP], keep.rearrange("p n o -> p (n o)"))
    ps_l = psa.tile([128, 512], dt.float32, tag="acc")
    nc.tensor.matmul(ps_l[0:16, 0:1], stats, ones_col, start=True, stop=True)
    lsum = st.tile([16, 1], dt.float32)
    nc.vector.tensor_copy(lsum, ps_l[0:16, 0:1])

    # ---------------- hinge term (identical on every core) -----------------
    zrow = qzs[:, 160:180]
    s1row = qzs[:, 180:200]
    zinv = st.tile([1, C], dt.float32)
    nc.vector.reciprocal(zinv, zrow)
    dcs = st.tile([1, C], dt.float32)
    nc.vector.tensor_mul(dcs, s1row, zinv)
    hv = st.tile([1, C], dt.float32)
    nc.vector.tensor_mul(hv, labrow_f, dcs)
    nc.scalar.activation(hv, hv, Act.Relu, bias=1.0, scale=-1.0)  # relu(1-lab*dcs)
    h = st.tile([1, 1], dt.float32)
    nc.vector.reduce_sum(h, hv, axis=AX.X)

    nc.sync.dma_start(out[0:1], h[:, 0])
    nc.sync.dma_start(out[1:17], lsum[:, 0])

    for pool in (psc, dp, psa, pss, pst, stp, st, const):
        pool.release()


def build_program(NS=512, F=4096, n_cores=8):
    nc = bacc.Bacc(
        "TRN2", target_bir_lowering=False, debug=False, num_devices=n_cores
    )
    KT = F // 128
    roi = nc.dram_tensor("roi", [128, KT, NS], dt.bfloat16, kind="ExternalInput").ap()
    frm = nc.dram_tensor("frm", [128, KT, NS], dt.bfloat16, kind="ExternalInput").ap()
    ctxm = nc.dram_tensor("ctxm", [128, KT, NS], dt.bfloat16, kind="ExternalInput").ap()
    w_a = nc.dram_tensor("w_a", [128, KT, CW], dt.bfloat16, kind="ExternalInput").ap()
    w_d2 = nc.dram_tensor("w_d2", [128, KT, 2 * C], dt.bfloat16, kind="ExternalInput").ap()
    w_r2 = nc.dram_tensor("w_r2", [128, KT, CR], dt.bfloat16, kind="ExternalInput").ap()
    b_a = nc.dram_tensor("b_a", [CW], dt.float32, kind="ExternalInput").ap()
    b_r2 = nc.dram_tensor("b_r2", [CR], dt.float32, kind="ExternalInput").ap()
    bxw = nc.dram_tensor("bxw", [128, NS // 128, 5], dt.float32, kind="ExternalInput").ap()
    isw = nc.dram_tensor("isw", [NS], dt.float32, kind="ExternalInput").ap()
    lab = nc.dram_tensor("lab", [1, C], dt.int32, kind="ExternalInput").ap()
    out = nc.dram_tensor("out", [17], dt.float32, kind="ExternalOutput").ap()
    aps = (roi, frm, ctxm, w_a, w_d2, w_r2, b_a, b_r2, bxw, isw, lab, out)
    with tile.TileContext(nc) as tc:
        _emit(nc, tc, aps, NS, F, n_cores)
    nc.compile()
    return nc


def _pack_fc7(a_t_bf16, sl, F):
    # [F, NS] bf16 slice -> [128, KT, NS] with contiguous per-partition runs
    return np.ascontiguousarray(
        a_t_bf16[:, sl].reshape(F // 128, 128, -1).transpose(1, 0, 2))


def _pack_w(w, cols):
    F = w.shape[0]
    return np.ascontiguousarray(
        w.astype(ml_dtypes.bfloat16).reshape(F // 128, 128, cols).transpose(1, 0, 2))


def make_in_maps(inputs, NS, n_cores):
    f32 = np.float32
    bf16 = ml_dtypes.bfloat16
    w_a = _pack_w(np.concatenate(
        [np.asarray(inputs["W_cls"], f32), np.asarray(inputs["W_r1"], f32)], axis=1), CW)
    wd = np.asarray(inputs["W_det"], f32)
    w_d2 = _pack_w(np.concatenate([wd, -wd], axis=1), 2 * C)
    w_r2 = _pack_w(np.asarray(inputs["W_r2"], f32), CR)
    b_a = np.ascontiguousarray(np.concatenate(
        [np.asarray(inputs["b_cls"]), np.asarray(inputs["b_r1"])]), f32)
    b_r2 = np.ascontiguousarray(np.asarray(inputs["b_r2"]), f32)
    boxes = np.asarray(inputs["ss_boxes"], f32)[:, 1:5]
    iswf = np.asarray(inputs["IS_weight"], f32)[:, 0]
    lab = np.ascontiguousarray(np.asarray(inputs["image_level_label"]), np.int32)
    roi = np.asarray(inputs["fc7_roi"], f32).T.astype(bf16)
    frm = np.asarray(inputs["fc7_frame"], f32).T.astype(bf16)
    ctxm = np.asarray(inputs["fc7_context"], f32).T.astype(bf16)
    F = roi.shape[0]
    NB = NS // 128

    in_maps = []
    for c in range(n_cores):
        sl = slice(c * NS, (c + 1) * NS)
        bsh = boxes[sl].reshape(NB, 128, 4).transpose(1, 0, 2)
        ish = iswf[sl].reshape(NB, 128).T[:, :, None]
        bxw = np.ascontiguousarray(np.concatenate([bsh, ish], axis=2), f32)
        in_maps.append({
            "roi": _pack_fc7(roi, sl, F),
            "frm": _pack_fc7(frm, sl, F),
            "ctxm": _pack_fc7(ctxm, sl, F),
            "w_a": w_a, "w_d2": w_d2, "w_r2": w_r2,
            "b_a": b_a, "b_r2": b_r2,
            "bxw": bxw, "isw": np.ascontiguousarray(iswf[sl]), "lab": lab,
        })
    return in_maps


_PROG_CACHE = {}


def _get_prog(NS, F, n_cores):
    key = (NS, F, n_cores)
    if key not in _PROG_CACHE:
        _PROG_CACHE[key] = build_program(NS, F, n_cores)
    return _PROG_CACHE[key]


def finish(results, n_cores=8):
    # host-side gather/unshard: combine the per-core partial sums
    parts = np.stack([np.asarray(results[i]["out"], np.float64).reshape(17)
                      for i in range(n_cores)])
    h = parts[0, 0]
    wl = parts[:, 1:9].sum(axis=0)      # per (b, s=idx%2) weighted log-probs
    kp = parts[:, 9:17].sum(axis=0)     # per (b, s) keep counts
    rl1 = -wl[0::2].sum() / kp[0::2].sum()
    rl2 = -wl[1::2].sum() / kp[1::2].sum()
    return np.float32(h / C + 0.1 * rl1 + 0.1 * rl2)


def kernel(**inputs):
    n_cores = 8
    N, F = inputs["fc7_roi"].shape
    NS = N // n_cores
    prog = _get_prog(NS, F, n_cores)
    in_maps = make_in_maps(inputs, NS, n_cores)
    res = run_bass_kernel_spmd(prog, in_maps, list(range(n_cores))).results
    return finish(res, n_cores)


# revision 14
# speedup vs baseline: 1.5609x; 1.0208x over previous
# Trainium2 Bass kernel for nn_Network_515396076038 (nms_detection / OICR-style loss).
#
# v3 strategy (8 NeuronCores, data-parallel over the N=4096 proposals):
#   - Inputs stream in bf16 (host-cast): ~12.9 MB/core, PE runs bf16 matmuls
#     at 1 cycle/row. fc7 shards are host-packed to [128, KT, NS] so every
#     big DMA is 128 contiguous per-partition runs. Small DMAs are queued
#     after the first roi chunk so the PE starts ASAP.
#   - det head: frame-context subtract on DVE/GpSimd (idle during the GEMM),
#     one det GEMM instead of two.
#   - All post-GEMM elementwise stats run ROI-MAJOR ([128, NB, *] tiles, full
#     128 DVE lanes) after tiny PE transposes; per-class sums use matmuls,
#     the per-class argmax max uses transpose + free-dim reduce. Candidate
#     box AREAS ride the same sel-mask gather matmul (5th lhsT column) and
#     ship in the AllGather payload, shortening the post-collective chain.
#   - The r2 refine head GEMM + its log-softmax run AFTER the AllGather
#     trigger (t_roi stays resident in SBUF), filling collective latency.
#   - Log-softmax needs no max-subtraction (|scores| < ~4): lp = x - ln(sum exp x).
#   - One AllGather total. The final loss reduction is done on host from
#     per-core partial sums (the gather/unshard step).
import sys

for _p in ("/opt/trn_rl_repo",):
    if _p not in sys.path:
        sys.path.append(_p)

import ml_dtypes
import numpy as np

import concourse.bass as bass
import concourse.bass_isa as bass_isa
import concourse.mybir as mybir
import concourse.tile as tile
from concourse import bacc
from concourse.bass_utils import run_bass_kernel_spmd
from concourse.masks import make_identity

dt = mybir.dt
Alu = mybir.AluOpType
Act = mybir.ActivationFunctionType
AX = mybir.AxisListType

C = 20       # foreground classes
CR = C + 1   # refine head classes (background + C)
CW = C + CR  # stacked critical-path roi heads: cls | r1 = 41


def _emit(nc, tc, aps, NS, F, n_cores):
    NB = NS // 128   # 4 roi blocks
    KT = F // 128    # 32 contraction slices
    KH = KT // 2     # roi chunk size
    KQ = KT // 4     # frm/ctx chunk size
    NP = NB * 2      # (block, supervision) pairs, index b*2+s
    group = [list(range(n_cores))]
    GW = 281  # AllGather row: vm[41] | boxes+areas[200] | z[20] | s1[20]

    (roi, frm, ctxm, w_a, w_det, w_r2, b_a, b_r2, bxw, lab, out) = aps

    const = tc.alloc_tile_pool(name="const", bufs=1)
    st = tc.alloc_tile_pool(name="st", bufs=1)
    pst = tc.alloc_tile_pool(name="pst", bufs=2, space="PSUM")
    pss = tc.alloc_tile_pool(name="pss", bufs=2, space="PSUM")
    psa = tc.alloc_tile_pool(name="psa", bufs=1, space="PSUM")
    dp = tc.alloc_tile_pool(name="dp", bufs=1, space="DRAM")
    psc = tc.alloc_tile_pool(name="psc", bufs=1, space="PSUM")

    # ---------------- DMA issue order = arrival order ----------------------
    w_a_sb = const.tile([128, KT, CW], dt.bfloat16)
    nc.sync.dma_start(w_a_sb, w_a)
    t_roi1 = st.tile([128, KH, NS], dt.bfloat16)
    nc.sync.dma_start(t_roi1, roi[:, 0:KH, :])
    t_roi2 = st.tile([128, KH, NS], dt.bfloat16)
    nc.sync.dma_start(t_roi2, roi[:, KH:KT, :])
    w_det_sb = const.tile([128, KT, C], dt.bfloat16)
    nc.sync.dma_start(w_det_sb, w_det)
    b_a_sb = const.tile([CW, 1], dt.float32)
    nc.sync.dma_start(b_a_sb, b_a[:, None])
    b_r2_sb = const.tile([CR, 1], dt.float32)
    nc.sync.dma_start(b_r2_sb, b_r2[:, None])
    bxw_sb = st.tile([128, NB, 5], dt.float32)
    nc.sync.dma_start(bxw_sb, bxw)
    labrow_i = st.tile([1, C], dt.int32)
    nc.sync.dma_start(labrow_i, lab)
    t_frm = []
    t_ctx = []
    for q in range(4):
        qf = st.tile([128, KQ, NS], dt.bfloat16, tag=f"frm{q}")
        nc.sync.dma_start(qf, frm[:, q * KQ:(q + 1) * KQ, :])
        t_frm.append(qf)
        qc = st.tile([128, KQ, NS], dt.bfloat16, tag=f"ctx{q}")
        nc.sync.dma_start(qc, ctxm[:, q * KQ:(q + 1) * KQ, :])
        t_ctx.append(qc)
    w_r2_sb = const.tile([128, KT, CR], dt.bfloat16)
    nc.sync.dma_start(w_r2_sb, w_r2)
    boxes_nat = bxw_sb[:, :, 0:4]
    isw_col = bxw_sb[:, :, 4:5]     # [128, NB, 1]

    # ---------------- constants ----------------
    ident = const.tile([128, 128], dt.float32)
    make_identity(nc, ident)
    ones_col = const.tile([128, 1], dt.float32)
    nc.vector.memset(ones_col, 1.0)
    ones_row = const.tile([1, 128], dt.float32)
    nc.vector.memset(ones_row, 1.0)
    iota_i = const.tile([128, CR], dt.int32)
    nc.gpsimd.iota(iota_i, pattern=[[1, CR]], base=0, channel_multiplier=0)
    iota_f = const.tile([128, CR], dt.float32)
    nc.vector.tensor_copy(iota_f, iota_i)
    iota_m1k = const.tile([128, C], dt.float32)
    nc.vector.tensor_scalar_add(iota_m1k, iota_f[:, :C], -1000.0)

    labrow_f = st.tile([1, C], dt.float32)
    nc.vector.tensor_copy(labrow_f, labrow_i)
    mask_row = st.tile([1, 2 * C], dt.float32)
    nc.vector.tensor_scalar(mask_row[:, 0:C], labrow_f, 1.0, None, Alu.is_equal)
    nc.vector.tensor_copy(mask_row[:, C:2 * C], mask_row[:, 0:C])

    # roi areas + boxes|area pack for the sel gather (early, off critical path)
    ab_all = st.tile([128, NB, 1], dt.float32)
    tw = st.tile([128, NB, 1], dt.float32)
    nc.vector.tensor_tensor(ab_all, bxw_sb[:, :, 2:3], bxw_sb[:, :, 0:1], Alu.subtract)
    nc.vector.tensor_scalar_add(ab_all, ab_all, 1.0)
    nc.vector.tensor_tensor(tw, bxw_sb[:, :, 3:4], bxw_sb[:, :, 1:2], Alu.subtract)
    nc.vector.tensor_scalar_add(tw, tw, 1.0)
    nc.vector.tensor_mul(ab_all, ab_all, tw)
    bxa = st.tile([128, NB, 5], dt.float32)
    nc.vector.tensor_copy(bxa[:, :, 0:4], boxes_nat)
    nc.vector.tensor_copy(bxa[:, :, 4:5], ab_all)

    # class mask broadcast down the partitions (filler, local-only)
    ps_m = pss.tile([128, 512], dt.float32, tag="mm")
    nc.tensor.matmul(ps_m[:, 0:2 * C], ones_row[0:1, :], mask_row, start=True, stop=True)
    maskP = st.tile([128, 2 * C], dt.float32)
    nc.vector.tensor_copy(maskP, ps_m[:, 0:2 * C])
    maskP_m1 = st.tile([128, 2 * C], dt.float32)
    nc.vector.tensor_scalar_add(maskP_m1, maskP, -1.0)

    # ---------------- main GEMM (bf16) -------------------------------------
    scoresA = psc.tile([128, NS], dt.float32)   # rows 0:CW = cls | r1
    scoresB = psc.tile([128, NS], dt.float32)   # rows 0:C  = det (frm - ctx)
    scoresR = psc.tile([128, NS], dt.float32)   # rows 0:CR = r2 (deferred)
    for k in range(KH):
        nc.tensor.matmul(scoresA[0:CW, :], w_a_sb[:, k, :], t_roi1[:, k, :],
                         start=(k == 0), stop=False)
    for k in range(KH):
        nc.tensor.matmul(scoresA[0:CW, :], w_a_sb[:, KH + k, :], t_roi2[:, k, :],
                         start=False, stop=(k == KH - 1))
    for q in range(4):
        dif = t_frm[q]   # frame-context, in place (frm never read by the PE)
        for k in range(KQ):
            eng = nc.vector if k % 2 == 0 else nc.gpsimd
            eng.tensor_sub(dif[:, k, :], t_frm[q][:, k, :], t_ctx[q][:, k, :])
        for k in range(KQ):
            kk = q * KQ + k
            nc.tensor.matmul(scoresB[0:C, :], w_det_sb[:, kk, :], dif[:, k, :],
                             start=(kk == 0), stop=(kk == KT - 1))

    # ---------------- roi-major stats (critical path to the AllGather) -----
    sA = st.tile([CW, NS], dt.float32)
    nc.scalar.activation(sA, scoresA[0:CW, :], Act.Identity, bias=b_a_sb)
    dE = st.tile([C, NS], dt.float32)            # exp(det), class-major
    nc.scalar.activation(dE, scoresB[0:C, :], Act.Exp)

    rm = st.tile([128, NB, CW], dt.float32)      # cls|r1 raw scores, roi-major
    ed = st.tile([128, NB, C], dt.float32)       # exp(det), roi-major
    for b in range(NB):
        bsl = bass.ts(b, 128)
        ptx = pst.tile([128, 64], dt.float32, tag="pt")
        nc.tensor.transpose(ptx[:, 0:CW], sA[:, bsl], ident[0:CW, 0:CW])
        nc.vector.tensor_copy(rm[:, b, :], ptx[:, 0:CW])
        ptd = pst.tile([128, 64], dt.float32, tag="pt")
        nc.tensor.transpose(ptd[:, 0:C], dE[:, bsl], ident[0:C, 0:C])
        nc.vector.tensor_copy(ed[:, b, :], ptd[:, 0:C])

    e = st.tile([128, NB, CW], dt.float32)       # exp(cls)|exp(r1)
    nc.scalar.activation(e, rm, Act.Exp)
    ec = e[:, :, 0:C]
    er = e[:, :, C:CW]
    scls = st.tile([128, NB, 1], dt.float32)
    nc.vector.reduce_sum(scls, ec, axis=AX.X)
    sr1 = st.tile([128, NB, 1], dt.float32)
    nc.vector.reduce_sum(sr1, er, axis=AX.X)
    rb1 = st.tile([128, NB, 1], dt.float32)
    nc.vector.reciprocal(rb1, scls)
    nc.vector.tensor_mul(rb1, rb1, isw_col)
    rb2 = st.tile([128, NB, 1], dt.float32)
    nc.vector.reciprocal(rb2, sr1)
    nc.vector.tensor_mul(rb2, rb2, isw_col)

    pq = st.tile([128, NB, CW], dt.float32)      # p1(20) | q2(21)
    nc.vector.tensor_mul(pq[:, :, 0:C], ec, ed)
    nc.vector.tensor_tensor(
        pq[:, :, 0:C], pq[:, :, 0:C], rb1.to_broadcast([128, NB, C]), Alu.mult)
    nc.vector.tensor_tensor(
        pq[:, :, C:CW], er, rb2.to_broadcast([128, NB, CR]), Alu.mult)

    # z / s1 partial sums: [1, 40] row via ones-matmul over rois
    zsp = st.tile([128, NB, 2 * C], dt.float32)  # exp(det) | cls*exp(det)
    nc.vector.tensor_copy(zsp[:, :, 0:C], ed)
    nc.vector.tensor_mul(zsp[:, :, C:2 * C], rm[:, :, 0:C], ed)
    ps_z = pss.tile([128, 512], dt.float32, tag="mm")
    for b in range(NB):
        nc.tensor.matmul(ps_z[0:1, 0:2 * C], ones_col, zsp[:, b, :],
                         start=(b == 0), stop=(b == NB - 1))
    zrow = st.tile([1, 2 * C], dt.float32)
    nc.vector.tensor_copy(zrow, ps_z[0:1, 0:2 * C])

    # per-class max over rois: free-dim max over blocks, transpose, reduce
    pmax = st.tile([128, CW], dt.float32)
    nc.vector.tensor_reduce(pmax, pq.rearrange("p b c -> p c b"),
                            axis=AX.X, op=Alu.max)
    ps_t = pss.tile([128, 512], dt.float32, tag="mm")
    nc.tensor.transpose(ps_t[0:CW, 0:128], pmax, ident)
    vt = st.tile([CW, 128], dt.float32)
    nc.vector.tensor_copy(vt, ps_t[0:CW, 0:128])
    vm = st.tile([CW, 1], dt.float32)
    nc.vector.reduce_max(vm, vt, axis=AX.X)
    ps_vr = pst.tile([128, 64], dt.float32, tag="pt")
    nc.tensor.transpose(ps_vr[0:1, 0:CW], vm, ident[0:CW, 0:CW])
    vmr = st.tile([1, CW], dt.float32)
    nc.vector.tensor_copy(vmr, ps_vr[0:1, 0:CW])
    ps_vb = pss.tile([128, 512], dt.float32, tag="mm")
    nc.tensor.matmul(ps_vb[:, 0:CW], ones_row[0:1, :], vmr, start=True, stop=True)
    vmP = st.tile([128, CW], dt.float32)
    nc.vector.tensor_copy(vmP, ps_vb[:, 0:CW])

    sel = st.tile([128, NB, CW], dt.float32)
    nc.vector.tensor_tensor(
        sel, pq, vmP[:, None, :].to_broadcast([128, NB, CW]), Alu.is_equal)
    psq = psa.tile([128, 512], dt.float32, tag="acc")
    for b in range(NB):
        nc.tensor.matmul(psq[0:5, 0:CW], bxa[:, b, :], sel[:, b, :],
                         start=(b == 0), stop=(b == NB - 1))
    bc_sb = st.tile([5, 2 * C], dt.float32)      # winner boxes+areas, s-paired
    nc.vector.tensor_copy(bc_sb[:, 0:C], psq[0:5, 0:C])
    nc.vector.tensor_copy(bc_sb[:, C:2 * C], psq[0:5, CR:CW])   # skip bg col

    # ---------------- G1: AllGather of all cross-core state ----------------
    g1_in = dp.tile([GW], dt.float32)
    g1_out = dp.tile([n_cores * GW], dt.float32)
    nc.scalar.dma_start(g1_in[0:CW], vm[:, 0])
    nc.sync.dma_start(g1_in[CW:241], bc_sb)
    nc.scalar.dma_start(g1_in[241:281], zrow)
    nc.gpsimd.collective_compute(
        "AllGather", Alu.bypass, replica_groups=group,
        ins=[g1_in.opt()], outs=[g1_out.opt()],
    )

    # ---- collective-latency filler: r2 GEMM + both heads' log-softmax -----
    for k in range(KH):
        nc.tensor.matmul(scoresR[0:CR, :], w_r2_sb[:, k, :], t_roi1[:, k, :],
                         start=(k == 0), stop=False)
    for k in range(KH):
        nc.tensor.matmul(scoresR[0:CR, :], w_r2_sb[:, KH + k, :], t_roi2[:, k, :],
                         start=False, stop=(k == KH - 1))
    r2c = st.tile([CR, NS], dt.float32)
    nc.scalar.activation(r2c, scoresR[0:CR, :], Act.Identity, bias=b_r2_sb)
    r2m = st.tile([128, NB, CR], dt.float32)
    for b in range(NB):
        bsl = bass.ts(b, 128)
        ptr = pst.tile([128, 64], dt.float32, tag="pt")
        nc.tensor.transpose(ptr[:, 0:CR], r2c[:, bsl], ident[0:CR, 0:CR])
        nc.vector.tensor_copy(r2m[:, b, :], ptr[:, 0:CR])
    er2 = st.tile([128, NB, CR], dt.float32)
    nc.scalar.activation(er2, r2m, Act.Exp)
    sr2 = st.tile([128, NB, 1], dt.float32)
    nc.vector.reduce_sum(sr2, er2, axis=AX.X)

    # log-probs for both supervisions: x - ln(sum exp x); pairs (b, s)
    xs = st.tile([128, NB, 2, CR], dt.float32)
    ln1 = st.tile([128, NB, 1], dt.float32)
    nc.scalar.activation(ln1, sr1, Act.Ln)
    nc.vector.tensor_tensor(
        xs[:, :, 0, :], rm[:, :, C:CW],
        ln1.to_broadcast([128, NB, CR]), Alu.subtract)
    ln2 = st.tile([128, NB, 1], dt.float32)
    nc.scalar.activation(ln2, sr2, Act.Ln)
    nc.vector.tensor_tensor(
        xs[:, :, 1, :], r2m, ln2.to_broadcast([128, NB, CR]), Alu.subtract)

    # ---------------- G1 readback + cross-core combine ----------------
    g_sb = st.tile([n_cores, GW], dt.float32)
    nc.sync.dma_start(g_sb, g1_out.rearrange("(r w) -> r w", r=n_cores))
    vmx = st.tile([n_cores, CW], dt.float32)
    nc.gpsimd.partition_all_reduce(
        vmx, g_sb[:, 0:CW], channels=n_cores, reduce_op=bass_isa.ReduceOp.max
    )
    selc = st.tile([n_cores, CW], dt.float32)
    nc.vector.tensor_tensor(selc, g_sb[:, 0:CW], vmx, Alu.is_equal)
    masked = st.tile([n_cores, 240], dt.float32)
    mview = masked[:, 0:200].rearrange("p (co s c) -> p co s c", co=5, s=2)
    gview = g_sb[:, CW:241].rearrange("p (co s c) -> p co s c", co=5, s=2)
    nc.vector.tensor_tensor(
        mview[:, :, 0, :], gview[:, :, 0, :],
        selc[:, None, 0:C].to_broadcast([n_cores, 5, C]), Alu.mult,
    )
    nc.vector.tensor_tensor(
        mview[:, :, 1, :], gview[:, :, 1, :],
        selc[:, None, CR:CW].to_broadcast([n_cores, 5, C]), Alu.mult,
    )
    nc.vector.tensor_copy(masked[:, 200:240], g_sb[:, 241:281])
    ps_qr = pss.tile([128, 512], dt.float32, tag="mm")
    nc.tensor.matmul(ps_qr[0:1, 0:240], ones_col[0:n_cores, :], masked,
                     start=True, stop=True)
    qzs = st.tile([1, 240], dt.float32)
    nc.vector.tensor_copy(qzs, ps_qr[0:1, 0:240])

    ps_q = pss.tile([128, 512], dt.float32, tag="mm")
    nc.tensor.matmul(ps_q[:, 0:200], ones_row[0:1, :], qzs[:, 0:200],
                     start=True, stop=True)
    QA = st.tile([128, 200], dt.float32)   # x1|y1|x2|y2|area, each [s,c] paired
    nc.vector.tensor_copy(QA, ps_q[:, 0:200])

    # ---------------- batched IoU / assignment / loss ----------------------
    W2 = 2 * C
    def qb(lo):   # query coord block [128, 1, 2C] -> [128, NB, 2C]
        return QA[:, None, lo:lo + W2].to_broadcast([128, NB, W2])
    def bb(i):    # per-block box coord [128, NB, 1] -> [128, NB, 2C]
        return boxes_nat[:, :, i:i + 1].to_broadcast([128, NB, W2])

    aqab = st.tile([128, NB, W2], dt.float32)    # area_q + area_b
    nc.vector.tensor_tensor(aqab, qb(160), ab_all.to_broadcast([128, NB, W2]), Alu.add)
    xi1 = st.tile([128, NB, W2], dt.float32)
    nc.vector.tensor_tensor(xi1, qb(0), bb(0), Alu.max)
    yi1 = st.tile([128, NB, W2], dt.float32)
    nc.vector.tensor_tensor(yi1, qb(40), bb(1), Alu.max)
    xi2 = st.tile([128, NB, W2], dt.float32)
    nc.vector.tensor_tensor(xi2, qb(80), bb(2), Alu.min)
    yi2 = st.tile([128, NB, W2], dt.float32)
    nc.vector.tensor_tensor(yi2, qb(120), bb(3), Alu.min)
    nc.vector.tensor_tensor(xi2, xi2, xi1, Alu.subtract)
    nc.vector.tensor_scalar(xi2, xi2, 1.0, 0.0, Alu.add, Alu.max)   # iw
    nc.vector.tensor_tensor(yi2, yi2, yi1, Alu.subtract)
    nc.vector.tensor_scalar(yi2, yi2, 1.0, 0.0, Alu.add, Alu.max)   # ih
    inter = st.tile([128, NB, W2], dt.float32)
    nc.vector.tensor_mul(inter, xi2, yi2)
    un = st.tile([128, NB, W2], dt.float32)
    nc.vector.tensor_tensor(un, aqab, inter, Alu.subtract)
    nc.vector.reciprocal(un, un)
    ov = st.tile([128, NB, W2], dt.float32)
    nc.vector.tensor_mul(ov, inter, un)
    nc.vector.tensor_tensor(
        ov, ov, maskP[:, None, :].to_broadcast([128, NB, W2]), Alu.mult)
    nc.vector.tensor_tensor(
        ov, ov, maskP_m1[:, None, :].to_broadcast([128, NB, W2]), Alu.add)

    ovp = ov.rearrange("p b (s c) -> p (b s) c", s=2)   # [128, NP, C]
    mo = st.tile([128, NP, 1], dt.float32)
    nc.vector.reduce_max(mo, ovp, axis=AX.X)
    meq = st.tile([128, NP, C], dt.float32)
    nc.vector.tensor_tensor(
        meq, ovp, mo.to_broadcast([128, NP, C]), Alu.is_equal)
    nc.vector.tensor_tensor(
        meq, meq, iota_m1k[:, None, :].to_broadcast([128, NP, C]), Alu.mult)
    gt = st.tile([128, NP, 1], dt.float32)
    nc.vector.tensor_reduce(gt, meq, axis=AX.X, op=Alu.min)
    nc.vector.tensor_scalar_add(gt, gt, 1001.0)          # argmax + 1

    fg = st.tile([128, NP, 1], dt.float32)
    nc.vector.tensor_scalar(fg, mo, 0.5, None, Alu.is_gt)
    keep = st.tile([128, NP, 1], dt.float32)
    nc.vector.tensor_scalar(keep, mo, 0.1, None, Alu.is_ge)
    col = st.tile([128, NP, 1], dt.float32)
    nc.vector.tensor_mul(col, gt, fg)                    # fg ? argmax+1 : 0
    oh = st.tile([128, NP, CR], dt.float32)
    nc.vector.tensor_tensor(
        oh, iota_f[:, None, :].to_broadcast([128, NP, CR]),
        col.to_broadcast([128, NP, CR]), Alu.is_equal)
    nc.vector.tensor_mul(oh, oh, xs.rearrange("p b s c -> p (b s) c"))
    lpsel = st.tile([128, NP, 1], dt.float32)
    nc.vector.reduce_sum(lpsel, oh, axis=AX.X)

    stats = st.tile([128, 16], dt.float32)               # wl[8] | keep[8]
    wv = stats[:, 0:NP].rearrange("p (b s) -> p b s", s=2)
    kv = keep.rearrange("p (b s) o -> p b (s o)", s=2)   # [128, NB, 2]
    nc.vector.tensor_tensor(wv, kv, isw_col.to_broadcast([128, NB, 2]), Alu.mult)
    nc.vector.tensor_mul(
        stats[:, 0:NP], stats[:, 0:NP],
        lpsel.rearrange("p n o -> p (n o)"))
    nc.vector.tensor_copy(stats[:, NP:2 * NP], keep.rearrange("p n o -> p (n o)"))
    ps_l = psa.tile([128, 512], dt.float32, tag="acc")
    nc.tensor.matmul(ps_l[0:16, 0:1], stats, ones_col, start=True, stop=True)
    lsum = st.tile([16, 1], dt.float32)
    nc.vector.tensor_copy(lsum, ps_l[0:16, 0:1])

    # ---------------- hinge term (identical on every core) -----------------
    zr = qzs[:, 200:220]
    s1r = qzs[:, 220:240]
    zinv = st.tile([1, C], dt.float32)
    nc.vector.reciprocal(zinv, zr)
    dcs = st.tile([1, C], dt.float32)
    nc.vector.tensor_mul(dcs, s1r, zinv)
    hv = st.tile([1, C], dt.float32)
    nc.vector.tensor_mul(hv, labrow_f, dcs)
    nc.scalar.activation(hv, hv, Act.Relu, bias=1.0, scale=-1.0)  # relu(1-lab*dcs)
    h = st.tile([1, 1], dt.float32)
    nc.vector.reduce_sum(h, hv, axis=AX.X)

    nc.scalar.dma_start(out[0:1], h[:, 0])
    nc.sync.dma_start(out[1:17], lsum[:, 0])

    for pool in (psc, dp, psa, pss, pst, st, const):
        pool.release()


def build_program(NS=512, F=4096, n_cores=8):
    nc = bacc.Bacc(
        "TRN2", target_bir_lowering=False, debug=False, num_devices=n_cores
    )
    KT = F // 128
    roi = nc.dram_tensor("roi", [128, KT, NS], dt.bfloat16, kind="ExternalInput").ap()
    frm = nc.dram_tensor("frm", [128, KT, NS], dt.bfloat16, kind="ExternalInput").ap()
    ctxm = nc.dram_tensor("ctxm", [128, KT, NS], dt.bfloat16, kind="ExternalInput").ap()
    w_a = nc.dram_tensor("w_a", [128, KT, CW], dt.bfloat16, kind="ExternalInput").ap()
    w_det = nc.dram_tensor("w_det", [128, KT, C], dt.bfloat16, kind="ExternalInput").ap()
    w_r2 = nc.dram_tensor("w_r2", [128, KT, CR], dt.bfloat16, kind="ExternalInput").ap()
    b_a = nc.dram_tensor("b_a", [CW], dt.float32, kind="ExternalInput").ap()
    b_r2 = nc.dram_tensor("b_r2", [CR], dt.float32, kind="ExternalInput").ap()
    bxw = nc.dram_tensor("bxw", [128, NS // 128, 5], dt.float32, kind="ExternalInput").ap()
    lab = nc.dram_tensor("lab", [1, C], dt.int32, kind="ExternalInput").ap()
    out = nc.dram_tensor("out", [17], dt.float32, kind="ExternalOutput").ap()
    aps = (roi, frm, ctxm, w_a, w_det, w_r2, b_a, b_r2, bxw, lab, out)
    with tile.TileContext(nc) as tc:
        _emit(nc, tc, aps, NS, F, n_cores)
    nc.compile()
    return nc


def _pack_fc7(a_t_bf16, sl, F):
    # [F, NS] bf16 slice -> [128, KT, NS] with contiguous per-partition runs
    return np.ascontiguousarray(
        a_t_bf16[:, sl].reshape(F // 128, 128, -1).transpose(1, 0, 2))


def _pack_w(w, cols):
    F = w.shape[0]
    return np.ascontiguousarray(
        w.astype(ml_dtypes.bfloat16).reshape(F // 128, 128, cols).transpose(1, 0, 2))


def make_in_maps(inputs, NS, n_cores):
    f32 = np.float32
    bf16 = ml_dtypes.bfloat16
    w_a = _pack_w(np.concatenate(
        [np.asarray(inputs["W_cls"], f32), np.asarray(inputs["W_r1"], f32)], axis=1), CW)
    w_det = _pack_w(np.asarray(inputs["W_det"], f32), C)
    w_r2 = _pack_w(np.asarray(inputs["W_r2"], f32), CR)
    b_a = np.ascontiguousarray(np.concatenate(
        [np.asarray(inputs["b_cls"]), np.asarray(inputs["b_r1"])]), f32)
    b_r2 = np.ascontiguousarray(np.asarray(inputs["b_r2"]), f32)
    boxes = np.asarray(inputs["ss_boxes"], f32)[:, 1:5]
    iswf = np.asarray(inputs["IS_weight"], f32)[:, 0]
    lab = np.ascontiguousarray(np.asarray(inputs["image_level_label"]), np.int32)
    roi = np.asarray(inputs["fc7_roi"], f32).T.astype(bf16)
    frm = np.asarray(inputs["fc7_frame"], f32).T.astype(bf16)
    ctxm = np.asarray(inputs["fc7_context"], f32).T.astype(bf16)
    F = roi.shape[0]
    NB = NS // 128

    in_maps = []
    for c in range(n_cores):
        sl = slice(c * NS, (c + 1) * NS)
        bsh = boxes[sl].reshape(NB, 128, 4).transpose(1, 0, 2)
        ish = iswf[sl].reshape(NB, 128).T[:, :, None]
        bxw = np.ascontiguousarray(np.concatenate([bsh, ish], axis=2), f32)
        in_maps.append({
            "roi": _pack_fc7(roi, sl, F),
            "frm": _pack_fc7(frm, sl, F),
            "ctxm": _pack_fc7(ctxm, sl, F),
            "w_a": w_a, "w_det": w_det, "w_r2": w_r2,
            "b_a": b_a, "b_r2": b_r2,
            "bxw": bxw, "lab": lab,
        })
    return in_maps


_PROG_CACHE = {}


def _get_prog(NS, F, n_cores):
    key = (NS, F, n_cores)
    if key not in _PROG_CACHE:
        _PROG_CACHE[key] = build_program(NS, F, n_cores)
    return _PROG_CACHE[key]


def finish(results, n_cores=8):
    # host-side gather/unshard: combine the per-core partial sums
    parts = np.stack([np.asarray(results[i]["out"], np.float64).reshape(17)
                      for i in range(n_cores)])
    h = parts[0, 0]
    wl = parts[:, 1:9].sum(axis=0)      # per (b, s=idx%2) weighted log-probs
    kp = parts[:, 9:17].sum(axis=0)     # per (b, s) keep counts
    rl1 = -wl[0::2].sum() / kp[0::2].sum()
    rl2 = -wl[1::2].sum() / kp[1::2].sum()
    return np.float32(h / C + 0.1 * rl1 + 0.1 * rl2)


def kernel(**inputs):
    n_cores = 8
    N, F = inputs["fc7_roi"].shape
    NS = N // n_cores
    prog = _get_prog(NS, F, n_cores)
    in_maps = make_in_maps(inputs, NS, n_cores)
    res = run_bass_kernel_spmd(prog, in_maps, list(range(n_cores))).results
    return finish(res, n_cores)


# revision 18
# speedup vs baseline: 1.6852x; 1.0796x over previous
# Trainium2 Bass kernel for nn_Network_515396076038 (nms_detection / OICR-style loss).
#
# v3 strategy (8 NeuronCores, data-parallel over the N=4096 proposals):
#   - Inputs stream in bf16 (host-cast): ~12.9 MB/core, PE runs bf16 matmuls
#     at 1 cycle/row. fc7 shards are host-packed to [128, KT, NS] so every
#     big DMA is 128 contiguous per-partition runs. Small DMAs are queued
#     after the first roi chunk so the PE starts ASAP.
#   - det head: frame-context subtract on DVE/GpSimd (idle during the GEMM),
#     one det GEMM instead of two.
#   - All post-GEMM elementwise stats run ROI-MAJOR ([128, NB, *] tiles, full
#     128 DVE lanes) after tiny PE transposes; per-class sums use matmuls,
#     the per-class argmax max uses transpose + free-dim reduce. Candidate
#     box AREAS ride the same sel-mask gather matmul (5th lhsT column) and
#     ship in the AllGather payload, shortening the post-collective chain.
#   - The r2 refine head GEMM + its log-softmax run AFTER the AllGather
#     trigger (t_roi stays resident in SBUF), filling collective latency.
#   - Log-softmax needs no max-subtraction (|scores| < ~4): lp = x - ln(sum exp x).
#   - One AllGather total. The final loss reduction is done on host from
#     per-core partial sums (the gather/unshard step).
import sys

for _p in ("/opt/trn_rl_repo",):
    if _p not in sys.path:
        sys.path.append(_p)

import ml_dtypes
import numpy as np

import concourse.bass as bass
import concourse.bass_isa as bass_isa
import concourse.mybir as mybir
import concourse.tile as tile
from concourse import bacc
from concourse.bass_utils import run_bass_kernel_spmd
from concourse.masks import make_identity

dt = mybir.dt
Alu = mybir.AluOpType
Act = mybir.ActivationFunctionType
AX = mybir.AxisListType

C = 20       # foreground classes
CR = C + 1   # refine head classes (background + C)
CW = C + CR  # stacked critical-path roi heads: cls | r1 = 41


def _emit(nc, tc, aps, NS, F, n_cores):
    NB = NS // 128   # 4 roi blocks
    KT = F // 128    # 32 contraction slices
    KH = KT // 2     # roi chunk size
    KQ = KT // 4     # frm/ctx chunk size
    NP = NB * 2      # (block, supervision) pairs, index b*2+s
    group = [list(range(n_cores))]
    GW = 281  # AllGather row: vm[41] | boxes+areas[200] | z[20] | s1[20]

    (roi, frm, ctxm, w_a, w_det, w_r2, b_a, b_r2, bxw, lab, out) = aps

    const = tc.alloc_tile_pool(name="const", bufs=1)
    st = tc.alloc_tile_pool(name="st", bufs=1)
    pst = tc.alloc_tile_pool(name="pst", bufs=2, space="PSUM")
    pss = tc.alloc_tile_pool(name="pss", bufs=2, space="PSUM")
    psa = tc.alloc_tile_pool(name="psa", bufs=1, space="PSUM")
    dp = tc.alloc_tile_pool(name="dp", bufs=1, space="DRAM")
    psc = tc.alloc_tile_pool(name="psc", bufs=1, space="PSUM")

    # ---------------- DMA issue order = arrival order ----------------------
    w_a_sb = const.tile([128, KT, CW], dt.bfloat16)
    nc.sync.dma_start(w_a_sb, w_a)
    t_roi1 = st.tile([128, KH, NS], dt.bfloat16)
    nc.sync.dma_start(t_roi1, roi[:, 0:KH, :])
    t_roi2 = st.tile([128, KH, NS], dt.bfloat16)
    nc.sync.dma_start(t_roi2, roi[:, KH:KT, :])
    w_det_sb = const.tile([128, KT, 2 * C], dt.bfloat16)
    nc.sync.dma_start(w_det_sb, w_det)
    b_a_sb = const.tile([CW, 1], dt.float32)
    nc.sync.dma_start(b_a_sb, b_a[:, None])
    b_r2_sb = const.tile([CR, 1], dt.float32)
    nc.sync.dma_start(b_r2_sb, b_r2[:, None])
    bxw_sb = st.tile([128, NB, 5], dt.float32)
    nc.sync.dma_start(bxw_sb, bxw)
    labrow_i = st.tile([1, C], dt.int32)
    nc.sync.dma_start(labrow_i, lab)
    t_frm = []
    t_ctx = []
    for q in range(4):
        qf = st.tile([128, KQ, NS], dt.bfloat16, tag=f"frm{q}")
        nc.sync.dma_start(qf, frm[:, q * KQ:(q + 1) * KQ, :])
        t_frm.append(qf)
        qc = st.tile([128, KQ, NS], dt.bfloat16, tag=f"ctx{q}")
        nc.sync.dma_start(qc, ctxm[:, q * KQ:(q + 1) * KQ, :])
        t_ctx.append(qc)
    w_r2_sb = const.tile([128, KT, CR], dt.bfloat16)
    nc.sync.dma_start(w_r2_sb, w_r2)
    boxes_nat = bxw_sb[:, :, 0:4]
    isw_col = bxw_sb[:, :, 4:5]     # [128, NB, 1]

    # ---------------- constants ----------------
    ident = const.tile([128, 128], dt.float32)
    make_identity(nc, ident)
    ones_col = const.tile([128, 1], dt.float32)
    nc.vector.memset(ones_col, 1.0)
    ones_row = const.tile([1, 128], dt.float32)
    nc.vector.memset(ones_row, 1.0)
    iota_i = const.tile([128, CR], dt.int32)
    nc.gpsimd.iota(iota_i, pattern=[[1, CR]], base=0, channel_multiplier=0)
    iota_f = const.tile([128, CR], dt.float32)
    nc.vector.tensor_copy(iota_f, iota_i)
    iota_m1k = const.tile([128, C], dt.float32)
    nc.vector.tensor_scalar_add(iota_m1k, iota_f[:, :C], -1000.0)

    labrow_f = st.tile([1, C], dt.float32)
    nc.vector.tensor_copy(labrow_f, labrow_i)
    mask_row = st.tile([1, 2 * C], dt.float32)
    nc.vector.tensor_scalar(mask_row[:, 0:C], labrow_f, 1.0, None, Alu.is_equal)
    nc.vector.tensor_copy(mask_row[:, C:2 * C], mask_row[:, 0:C])

    # roi areas + boxes|area pack for the sel gather (early, off critical path)
    ab_all = st.tile([128, NB, 1], dt.float32)
    tw = st.tile([128, NB, 1], dt.float32)
    nc.vector.tensor_tensor(ab_all, bxw_sb[:, :, 2:3], bxw_sb[:, :, 0:1], Alu.subtract)
    nc.vector.tensor_scalar_add(ab_all, ab_all, 1.0)
    nc.vector.tensor_tensor(tw, bxw_sb[:, :, 3:4], bxw_sb[:, :, 1:2], Alu.subtract)
    nc.vector.tensor_scalar_add(tw, tw, 1.0)
    nc.vector.tensor_mul(ab_all, ab_all, tw)
    bxa = st.tile([128, NB, 5], dt.float32)
    nc.vector.tensor_copy(bxa[:, :, 0:4], boxes_nat)
    nc.vector.tensor_copy(bxa[:, :, 4:5], ab_all)

    # class mask broadcast down the partitions (filler, local-only)
    ps_m = pss.tile([128, 512], dt.float32, tag="mm")
    nc.tensor.matmul(ps_m[:, 0:2 * C], ones_row[0:1, :], mask_row, start=True, stop=True)
    maskP = st.tile([128, 2 * C], dt.float32)
    nc.vector.tensor_copy(maskP, ps_m[:, 0:2 * C])
    maskP_m1 = st.tile([128, 2 * C], dt.float32)
    nc.vector.tensor_scalar_add(maskP_m1, maskP, -1.0)

    # ---------------- main GEMM (bf16) -------------------------------------
    # det = frm@W + ctx@(-W): accumulate both into one PSUM bank; [W|-W] is
    # host-packed so no vector subtract is needed.
    scoresA = psc.tile([128, NS], dt.float32)   # rows 0:CW = cls | r1
    scoresB = psc.tile([128, NS], dt.float32)   # rows 0:C  = det (frm - ctx)
    for k in range(KH):
        nc.tensor.matmul(scoresA[0:CW, :], w_a_sb[:, k, :], t_roi1[:, k, :],
                         start=(k == 0), stop=False)
    for k in range(KH):
        nc.tensor.matmul(scoresA[0:CW, :], w_a_sb[:, KH + k, :], t_roi2[:, k, :],
                         start=False, stop=(k == KH - 1))
    for q in range(4):
        for k in range(KQ):
            kk = q * KQ + k
            nc.tensor.matmul(scoresB[0:C, :], w_det_sb[:, kk, 0:C], t_frm[q][:, k, :],
                             start=(kk == 0), stop=False)
        for k in range(KQ):
            kk = q * KQ + k
            nc.tensor.matmul(scoresB[0:C, :], w_det_sb[:, kk, C:2 * C], t_ctx[q][:, k, :],
                             start=False, stop=(kk == KT - 1))

    # ---------------- roi-major stats (critical path to the AllGather) -----
    sA = st.tile([CW, NS], dt.float32)
    nc.scalar.activation(sA, scoresA[0:CW, :], Act.Identity, bias=b_a_sb)
    dE = st.tile([C, NS], dt.float32)            # exp(det), class-major
    nc.scalar.activation(dE, scoresB[0:C, :], Act.Exp)

    rm = st.tile([128, NB, CW], dt.float32)      # cls|r1 raw scores, roi-major
    ed = st.tile([128, NB, C], dt.float32)       # exp(det), roi-major
    for b in range(NB):
        bsl = bass.ts(b, 128)
        ptx = pst.tile([128, 64], dt.float32, tag="pt")
        nc.tensor.transpose(ptx[:, 0:CW], sA[:, bsl], ident[0:CW, 0:CW])
        nc.vector.tensor_copy(rm[:, b, :], ptx[:, 0:CW])
        ptd = pst.tile([128, 64], dt.float32, tag="pt")
        nc.tensor.transpose(ptd[:, 0:C], dE[:, bsl], ident[0:C, 0:C])
        nc.vector.tensor_copy(ed[:, b, :], ptd[:, 0:C])

    e = st.tile([128, NB, CW], dt.float32)       # exp(cls)|exp(r1)
    nc.scalar.activation(e, rm, Act.Exp)
    ec = e[:, :, 0:C]
    er = e[:, :, C:CW]
    scls = st.tile([128, NB, 1], dt.float32)
    nc.vector.reduce_sum(scls, ec, axis=AX.X)
    sr1 = st.tile([128, NB, 1], dt.float32)
    nc.vector.reduce_sum(sr1, er, axis=AX.X)
    rb1 = st.tile([128, NB, 1], dt.float32)
    nc.vector.reciprocal(rb1, scls)
    nc.vector.tensor_mul(rb1, rb1, isw_col)
    rb2 = st.tile([128, NB, 1], dt.float32)
    nc.vector.reciprocal(rb2, sr1)
    nc.vector.tensor_mul(rb2, rb2, isw_col)

    pq = st.tile([128, NB, CW], dt.float32)      # p1(20) | q2(21)
    nc.vector.tensor_mul(pq[:, :, 0:C], ec, ed)
    nc.vector.tensor_tensor(
        pq[:, :, 0:C], pq[:, :, 0:C], rb1.to_broadcast([128, NB, C]), Alu.mult)
    nc.vector.tensor_tensor(
        pq[:, :, C:CW], er, rb2.to_broadcast([128, NB, CR]), Alu.mult)

    # z / s1 partial sums: [1, 40] row via ones-matmul over rois
    zsp = st.tile([128, NB, 2 * C], dt.float32)  # exp(det) | cls*exp(det)
    nc.vector.tensor_copy(zsp[:, :, 0:C], ed)
    nc.vector.tensor_mul(zsp[:, :, C:2 * C], rm[:, :, 0:C], ed)
    ps_z = pss.tile([128, 512], dt.float32, tag="mm")
    for b in range(NB):
        nc.tensor.matmul(ps_z[0:1, 0:2 * C], ones_col, zsp[:, b, :],
                         start=(b == 0), stop=(b == NB - 1))
    zrow = st.tile([1, 2 * C], dt.float32)
    nc.vector.tensor_copy(zrow, ps_z[0:1, 0:2 * C])

    # per-class max over rois: free-dim max over blocks, transpose, reduce
    pmax = st.tile([128, CW], dt.float32)
    nc.vector.tensor_reduce(pmax, pq.rearrange("p b c -> p c b"),
                            axis=AX.X, op=Alu.max)
    ps_t = pss.tile([128, 512], dt.float32, tag="mm")
    nc.tensor.transpose(ps_t[0:CW, 0:128], pmax, ident)
    vt = st.tile([CW, 128], dt.float32)
    nc.vector.tensor_copy(vt, ps_t[0:CW, 0:128])
    vm = st.tile([CW, 1], dt.float32)
    nc.vector.reduce_max(vm, vt, axis=AX.X)
    ps_vr = pst.tile([128, 64], dt.float32, tag="pt")
    nc.tensor.transpose(ps_vr[0:1, 0:CW], vm, ident[0:CW, 0:CW])
    vmr = st.tile([1, CW], dt.float32)
    nc.vector.tensor_copy(vmr, ps_vr[0:1, 0:CW])
    ps_vb = pss.tile([128, 512], dt.float32, tag="mm")
    nc.tensor.matmul(ps_vb[:, 0:CW], ones_row[0:1, :], vmr, start=True, stop=True)
    vmP = st.tile([128, CW], dt.float32)
    nc.vector.tensor_copy(vmP, ps_vb[:, 0:CW])

    sel = st.tile([128, NB, CW], dt.float32)
    nc.vector.tensor_tensor(
        sel, pq, vmP[:, None, :].to_broadcast([128, NB, CW]), Alu.is_equal)
    psq = psa.tile([128, 512], dt.float32, tag="acc")
    for b in range(NB):
        nc.tensor.matmul(psq[0:5, 0:CW], bxa[:, b, :], sel[:, b, :],
                         start=(b == 0), stop=(b == NB - 1))
    bc_sb = st.tile([5, 2 * C], dt.float32)      # winner boxes+areas, s-paired
    nc.vector.tensor_copy(bc_sb[:, 0:C], psq[0:5, 0:C])
    nc.vector.tensor_copy(bc_sb[:, C:2 * C], psq[0:5, CR:CW])   # skip bg col

    # ---------------- G1: AllGather of all cross-core state ----------------
    g1_in = dp.tile([GW], dt.float32)
    g1_out = dp.tile([n_cores * GW], dt.float32)
    nc.scalar.dma_start(g1_in[0:CW], vm[:, 0])
    nc.sync.dma_start(g1_in[CW:241], bc_sb)
    nc.scalar.dma_start(g1_in[241:281], zrow)
    nc.gpsimd.collective_compute(
        "AllGather", Alu.bypass, replica_groups=group,
        ins=[g1_in.opt()], outs=[g1_out.opt()],
    )

    # ---- collective-latency filler: r2 GEMM + both heads' log-softmax -----
    # scoresR reuses psq's PSUM buffer (psa pool): the WAR dependency keeps
    # the PE from hoisting the r2 GEMM in front of the argmax/gather chain.
    scoresR = psa.tile([128, 512], dt.float32, tag="acc")
    for k in range(KH):
        nc.tensor.matmul(scoresR[0:CR, :], w_r2_sb[:, k, :], t_roi1[:, k, :],
                         start=(k == 0), stop=False)
    for k in range(KH):
        nc.tensor.matmul(scoresR[0:CR, :], w_r2_sb[:, KH + k, :], t_roi2[:, k, :],
                         start=False, stop=(k == KH - 1))
    r2c = st.tile([CR, NS], dt.float32)
    nc.scalar.activation(r2c, scoresR[0:CR, :], Act.Identity, bias=b_r2_sb)
    r2m = st.tile([128, NB, CR], dt.float32)
    for b in range(NB):
        bsl = bass.ts(b, 128)
        ptr = pst.tile([128, 64], dt.float32, tag="pt")
        nc.tensor.transpose(ptr[:, 0:CR], r2c[:, bsl], ident[0:CR, 0:CR])
        nc.vector.tensor_copy(r2m[:, b, :], ptr[:, 0:CR])
    er2 = st.tile([128, NB, CR], dt.float32)
    nc.scalar.activation(er2, r2m, Act.Exp)
    sr2 = st.tile([128, NB, 1], dt.float32)
    nc.vector.reduce_sum(sr2, er2, axis=AX.X)

    # log-probs for both supervisions: x - ln(sum exp x); pairs (b, s)
    xs = st.tile([128, NB, 2, CR], dt.float32)
    ln1 = st.tile([128, NB, 1], dt.float32)
    nc.scalar.activation(ln1, sr1, Act.Ln)
    nc.vector.tensor_tensor(
        xs[:, :, 0, :], rm[:, :, C:CW],
        ln1.to_broadcast([128, NB, CR]), Alu.subtract)
    ln2 = st.tile([128, NB, 1], dt.float32)
    nc.scalar.activation(ln2, sr2, Act.Ln)
    nc.vector.tensor_tensor(
        xs[:, :, 1, :], r2m, ln2.to_broadcast([128, NB, CR]), Alu.subtract)

    # ---------------- G1 readback + cross-core combine ----------------
    g_sb = st.tile([n_cores, GW], dt.float32)
    nc.sync.dma_start(g_sb, g1_out.rearrange("(r w) -> r w", r=n_cores))
    vmx = st.tile([n_cores, CW], dt.float32)
    nc.gpsimd.partition_all_reduce(
        vmx, g_sb[:, 0:CW], channels=n_cores, reduce_op=bass_isa.ReduceOp.max
    )
    selc = st.tile([n_cores, CW], dt.float32)
    nc.vector.tensor_tensor(selc, g_sb[:, 0:CW], vmx, Alu.is_equal)
    masked = st.tile([n_cores, 240], dt.float32)
    mview = masked[:, 0:200].rearrange("p (co s c) -> p co s c", co=5, s=2)
    gview = g_sb[:, CW:241].rearrange("p (co s c) -> p co s c", co=5, s=2)
    nc.vector.tensor_tensor(
        mview[:, :, 0, :], gview[:, :, 0, :],
        selc[:, None, 0:C].to_broadcast([n_cores, 5, C]), Alu.mult,
    )
    nc.vector.tensor_tensor(
        mview[:, :, 1, :], gview[:, :, 1, :],
        selc[:, None, CR:CW].to_broadcast([n_cores, 5, C]), Alu.mult,
    )
    nc.vector.tensor_copy(masked[:, 200:240], g_sb[:, 241:281])
    ps_qr = pss.tile([128, 512], dt.float32, tag="mm")
    nc.tensor.matmul(ps_qr[0:1, 0:240], ones_col[0:n_cores, :], masked,
                     start=True, stop=True)
    qzs = st.tile([1, 240], dt.float32)
    nc.vector.tensor_copy(qzs, ps_qr[0:1, 0:240])

    ps_q = pss.tile([128, 512], dt.float32, tag="mm")
    nc.tensor.matmul(ps_q[:, 0:200], ones_row[0:1, :], qzs[:, 0:200],
                     start=True, stop=True)
    QA = st.tile([128, 200], dt.float32)   # x1|y1|x2|y2|area, each [s,c] paired
    nc.vector.tensor_copy(QA, ps_q[:, 0:200])

    # ---------------- batched IoU / assignment / loss ----------------------
    W2 = 2 * C
    def qb(lo):   # query coord block [128, 1, 2C] -> [128, NB, 2C]
        return QA[:, None, lo:lo + W2].to_broadcast([128, NB, W2])
    def bb(i):    # per-block box coord [128, NB, 1] -> [128, NB, 2C]
        return boxes_nat[:, :, i:i + 1].to_broadcast([128, NB, W2])

    aqab = st.tile([128, NB, W2], dt.float32)    # area_q + area_b
    nc.vector.tensor_tensor(aqab, qb(160), ab_all.to_broadcast([128, NB, W2]), Alu.add)
    xi1 = st.tile([128, NB, W2], dt.float32)
    nc.vector.tensor_tensor(xi1, qb(0), bb(0), Alu.max)
    yi1 = st.tile([128, NB, W2], dt.float32)
    nc.vector.tensor_tensor(yi1, qb(40), bb(1), Alu.max)
    xi2 = st.tile([128, NB, W2], dt.float32)
    nc.vector.tensor_tensor(xi2, qb(80), bb(2), Alu.min)
    yi2 = st.tile([128, NB, W2], dt.float32)
    nc.vector.tensor_tensor(yi2, qb(120), bb(3), Alu.min)
    nc.vector.tensor_tensor(xi2, xi2, xi1, Alu.subtract)
    nc.vector.tensor_scalar(xi2, xi2, 1.0, 0.0, Alu.add, Alu.max)   # iw
    nc.vector.tensor_tensor(yi2, yi2, yi1, Alu.subtract)
    nc.vector.tensor_scalar(yi2, yi2, 1.0, 0.0, Alu.add, Alu.max)   # ih
    inter = st.tile([128, NB, W2], dt.float32)
    nc.vector.tensor_mul(inter, xi2, yi2)
    un = st.tile([128, NB, W2], dt.float32)
    nc.vector.tensor_tensor(un, aqab, inter, Alu.subtract)
    nc.vector.reciprocal(un, un)
    ov = st.tile([128, NB, W2], dt.float32)
    nc.vector.tensor_mul(ov, inter, un)
    nc.vector.tensor_tensor(
        ov, ov, maskP[:, None, :].to_broadcast([128, NB, W2]), Alu.mult)
    nc.vector.tensor_tensor(
        ov, ov, maskP_m1[:, None, :].to_broadcast([128, NB, W2]), Alu.add)

    ovp = ov.rearrange("p b (s c) -> p (b s) c", s=2)   # [128, NP, C]
    mo = st.tile([128, NP, 1], dt.float32)
    nc.vector.reduce_max(mo, ovp, axis=AX.X)
    meq = st.tile([128, NP, C], dt.float32)
    nc.vector.tensor_tensor(
        meq, ovp, mo.to_broadcast([128, NP, C]), Alu.is_equal)
    nc.vector.tensor_tensor(
        meq, meq, iota_m1k[:, None, :].to_broadcast([128, NP, C]), Alu.mult)
    gt = st.tile([128, NP, 1], dt.float32)
    nc.vector.tensor_reduce(gt, meq, axis=AX.X, op=Alu.min)
    nc.vector.tensor_scalar_add(gt, gt, 1001.0)          # argmax + 1

    fg = st.tile([128, NP, 1], dt.float32)
    nc.vector.tensor_scalar(fg, mo, 0.5, None, Alu.is_gt)
    keep = st.tile([128, NP, 1], dt.float32)
    nc.vector.tensor_scalar(keep, mo, 0.1, None, Alu.is_ge)
    col = st.tile([128, NP, 1], dt.float32)
    nc.vector.tensor_mul(col, gt, fg)                    # fg ? argmax+1 : 0
    oh = st.tile([128, NP, CR], dt.float32)
    nc.vector.tensor_tensor(
        oh, iota_f[:, None, :].to_broadcast([128, NP, CR]),
        col.to_broadcast([128, NP, CR]), Alu.is_equal)
    nc.vector.tensor_mul(oh, oh, xs.rearrange("p b s c -> p (b s) c"))
    lpsel = st.tile([128, NP, 1], dt.float32)
    nc.vector.reduce_sum(lpsel, oh, axis=AX.X)

    stats = st.tile([128, 16], dt.float32)               # wl[8] | keep[8]
    wv = stats[:, 0:NP].rearrange("p (b s) -> p b s", s=2)
    kv = keep.rearrange("p (b s) o -> p b (s o)", s=2)   # [128, NB, 2]
    nc.vector.tensor_tensor(wv, kv, isw_col.to_broadcast([128, NB, 2]), Alu.mult)
    nc.vector.tensor_mul(
        stats[:, 0:NP], stats[:, 0:NP],
        lpsel.rearrange("p n o -> p (n o)"))
    nc.vector.tensor_copy(stats[:, NP:2 * NP], keep.rearrange("p n o -> p (n o)"))
    ps_l = psa.tile([128, 512], dt.float32, tag="acc")
    nc.tensor.matmul(ps_l[0:16, 0:1], stats, ones_col, start=True, stop=True)
    lsum = st.tile([16, 1], dt.float32)
    nc.vector.tensor_copy(lsum, ps_l[0:16, 0:1])

    # ---------------- hinge term (identical on every core) -----------------
    zr = qzs[:, 200:220]
    s1r = qzs[:, 220:240]
    zinv = st.tile([1, C], dt.float32)
    nc.vector.reciprocal(zinv, zr)
    dcs = st.tile([1, C], dt.float32)
    nc.vector.tensor_mul(dcs, s1r, zinv)
    hv = st.tile([1, C], dt.float32)
    nc.vector.tensor_mul(hv, labrow_f, dcs)
    nc.scalar.activation(hv, hv, Act.Relu, bias=1.0, scale=-1.0)  # relu(1-lab*dcs)
    h = st.tile([1, 1], dt.float32)
    nc.vector.reduce_sum(h, hv, axis=AX.X)

    nc.scalar.dma_start(out[0:1], h[:, 0])
    nc.sync.dma_start(out[1:17], lsum[:, 0])

    for pool in (psc, dp, psa, pss, pst, st, const):
        pool.release()


def build_program(NS=512, F=4096, n_cores=8):
    nc = bacc.Bacc(
        "TRN2", target_bir_lowering=False, debug=False, num_devices=n_cores
    )
    KT = F // 128
    roi = nc.dram_tensor("roi", [128, KT, NS], dt.bfloat16, kind="ExternalInput").ap()
    frm = nc.dram_tensor("frm", [128, KT, NS], dt.bfloat16, kind="ExternalInput").ap()
    ctxm = nc.dram_tensor("ctxm", [128, KT, NS], dt.bfloat16, kind="ExternalInput").ap()
    w_a = nc.dram_tensor("w_a", [128, KT, CW], dt.bfloat16, kind="ExternalInput").ap()
    w_det = nc.dram_tensor("w_det", [128, KT, 2 * C], dt.bfloat16, kind="ExternalInput").ap()
    w_r2 = nc.dram_tensor("w_r2", [128, KT, CR], dt.bfloat16, kind="ExternalInput").ap()
    b_a = nc.dram_tensor("b_a", [CW], dt.float32, kind="ExternalInput").ap()
    b_r2 = nc.dram_tensor("b_r2", [CR], dt.float32, kind="ExternalInput").ap()
    bxw = nc.dram_tensor("bxw", [128, NS // 128, 5], dt.float32, kind="ExternalInput").ap()
    lab = nc.dram_tensor("lab", [1, C], dt.int32, kind="ExternalInput").ap()
    out = nc.dram_tensor("out", [17], dt.float32, kind="ExternalOutput").ap()
    aps = (roi, frm, ctxm, w_a, w_det, w_r2, b_a, b_r2, bxw, lab, out)
    with tile.TileContext(nc) as tc:
        _emit(nc, tc, aps, NS, F, n_cores)
    nc.compile()
    return nc


def _pack_fc7(a_t_bf16, sl, F):
    # [F, NS] bf16 slice -> [128, KT, NS] with contiguous per-partition runs
    return np.ascontiguousarray(
        a_t_bf16[:, sl].reshape(F // 128, 128, -1).transpose(1, 0, 2))


def _pack_w(w, cols):
    F = w.shape[0]
    return np.ascontiguousarray(
        w.astype(ml_dtypes.bfloat16).reshape(F // 128, 128, cols).transpose(1, 0, 2))


def make_in_maps(inputs, NS, n_cores):
    f32 = np.float32
    bf16 = ml_dtypes.bfloat16
    w_a = _pack_w(np.concatenate(
        [np.asarray(inputs["W_cls"], f32), np.asarray(inputs["W_r1"], f32)], axis=1), CW)
    wd = np.asarray(inputs["W_det"], f32)
    w_det = _pack_w(np.concatenate([wd, -wd], axis=1), 2 * C)
    w_r2 = _pack_w(np.asarray(inputs["W_r2"], f32), CR)
    b_a = np.ascontiguousarray(np.concatenate(
        [np.asarray(inputs["b_cls"]), np.asarray(inputs["b_r1"])]), f32)
    b_r2 = np.ascontiguousarray(np.asarray(inputs["b_r2"]), f32)
    boxes = np.asarray(inputs["ss_boxes"], f32)[:, 1:5]
    iswf = np.asarray(inputs["IS_weight"], f32)[:, 0]
    lab = np.ascontiguousarray(np.asarray(inputs["image_level_label"]), np.int32)
    roi = np.asarray(inputs["fc7_roi"], f32).T.astype(bf16)
    frm = np.asarray(inputs["fc7_frame"], f32).T.astype(bf16)
    ctxm = np.asarray(inputs["fc7_context"], f32).T.astype(bf16)
    F = roi.shape[0]
    NB = NS // 128

    in_maps = []
    for c in range(n_cores):
        sl = slice(c * NS, (c + 1) * NS)
        bsh = boxes[sl].reshape(NB, 128, 4).transpose(1, 0, 2)
        ish = iswf[sl].reshape(NB, 128).T[:, :, None]
        bxw = np.ascontiguousarray(np.concatenate([bsh, ish], axis=2), f32)
        in_maps.append({
            "roi": _pack_fc7(roi, sl, F),
            "frm": _pack_fc7(frm, sl, F),
            "ctxm": _pack_fc7(ctxm, sl, F),
            "w_a": w_a, "w_det": w_det, "w_r2": w_r2,
            "b_a": b_a, "b_r2": b_r2,
            "bxw": bxw, "lab": lab,
        })
    return in_maps


_PROG_CACHE = {}


def _get_prog(NS, F, n_cores):
    key = (NS, F, n_cores)
    if key not in _PROG_CACHE:
        _PROG_CACHE[key] = build_program(NS, F, n_cores)
    return _PROG_CACHE[key]


def finish(results, n_cores=8):
    # host-side gather/unshard: combine the per-core partial sums
    parts = np.stack([np.asarray(results[i]["out"], np.float64).reshape(17)
                      for i in range(n_cores)])
    h = parts[0, 0]
    wl = parts[:, 1:9].sum(axis=0)      # per (b, s=idx%2) weighted log-probs
    kp = parts[:, 9:17].sum(axis=0)     # per (b, s) keep counts
    rl1 = -wl[0::2].sum() / kp[0::2].sum()
    rl2 = -wl[1::2].sum() / kp[1::2].sum()
    return np.float32(h / C + 0.1 * rl1 + 0.1 * rl2)


def kernel(**inputs):
    n_cores = 8
    N, F = inputs["fc7_roi"].shape
    NS = N // n_cores
    prog = _get_prog(NS, F, n_cores)
    in_maps = make_in_maps(inputs, NS, n_cores)
    res = run_bass_kernel_spmd(prog, in_maps, list(range(n_cores))).results
    return finish(res, n_cores)
